# revision 1
# baseline (speedup 1.0000x reference)
"""Trainium2 Bass kernel for block-causal (chunked) multi-head attention.

Computes, for x:[2,2048,1024], Wqkv:[3072,1024], Wout:[1024,1024]:
    qkv = x @ Wqkv.T ; per-head scaled scores; block-causal mask
    (causal OR same 64-chunk == full attention to all chunks <= own chunk);
    softmax; out = attn @ v ; y = out @ Wout.T

Sharding over 8 NeuronCores: data-parallel over batch (2) x tensor-parallel
over heads (16 heads -> 4 per core).  Each core projects q/k/v for its 4
heads, runs attention, and computes a partial output projection against its
256 columns of Wout; the host sums the 4 partials per batch element.

On-chip layout avoids all transposes: the host hands each core
  xT     [1024, 2048]  (x[b] transposed)
  wqkT   [1024, 512]   (Wqkv rows for its 4 heads' q,k -> transposed)
  wvT    [1024, 256]   (v rows transposed)
  woutT  [256, 1024]   (Wout columns for its head-slice, transposed)
Scores are computed transposed (S^T[tk, tq]) so that the attention matmul
P^T -> (attn @ V) needs no transposes, and the softmax denominator comes
for free from a ones-column appended to V.  The block-causal mask is
realized structurally: masked-out key blocks are simply never computed, and
the diagonal blocks use rectangular sub-views (chunk granularity 64).

Engines execute their instruction streams in order, so the emission is a
software pipeline over the 4 query tiles: the TensorE stream for the
(ScalarE-paced) attention of tile t is interleaved with "filler" matmul
chains -- the q/k/v projections of tile t+1 and the output projection of
tile t-1 -- keeping the PE busy through every exp dependency stall.
"""

import sys

if "/opt/trn_rl_repo" not in sys.path:
    sys.path.insert(0, "/opt/trn_rl_repo")

from collections import deque

import numpy as np

import concourse.bass as bass  # noqa: F401  (registers types)
import concourse.mybir as mybir
import concourse.tile as tile
from concourse import bacc
from concourse.bass_utils import run_bass_kernel_spmd

F32 = mybir.dt.float32
F32R = mybir.dt.float32r
EXP = mybir.ActivationFunctionType.Exp

B = 2
T = 2048
DIM = 1024
N_HEADS = 16
HD = 64
CHUNK = 64
H_PER_CORE = 4  # 16 heads / (8 cores / 2 batches)
QT = 512  # query tile (free dim of S^T matmuls)
KB = 128  # key block (contraction block of AV matmuls)
N_QT = T // QT  # 4
N_KB = T // KB  # 16
N_DIMB = DIM // 128  # 8 contraction blocks for the projections
SCALE = 1.0 / np.sqrt(HD)

SPLIT_Y = False
_CACHED_NC = None


def _emit(nc, tc, xT, wqkT, wvT, woT, y):
    po = tc.tile_pool  # shorthand

    with (
        po(name="persist", bufs=1) as pp,
        po(name="s_ps", bufs=2, space="PSUM") as sps,  # [128,1024] score slots
        po(name="mm_ps", bufs=2, space="PSUM") as mmps,  # [128,512] proj/y slots
        po(name="ot_ps", bufs=2, space="PSUM") as otps,  # [65,512] outT slots
        po(name="pbuf", bufs=4) as ppool,  # exp(S^T) tiles
        po(name="osbuf", bufs=2) as ospool,  # assembled normalized outT
        po(name="scbuf", bufs=2) as scpool,  # normalize scratch
        po(name="rbuf", bufs=2) as rpool,  # reciprocal denominators
        po(name="ybuf", bufs=3) as ypool,
    ):
        # ---- persistent SBUF tensors (chunked to keep deps fine-grained) ----
        xt = [
            [pp.tile([128, QT], F32R, tag=f"xt{k}_{c}", name=f"xt{k}_{c}") for c in range(N_QT)]
            for k in range(N_DIMB)
        ]
        wqk = [pp.tile([128, 512], F32R, tag=f"wqk{k}", name=f"wqk{k}") for k in range(N_DIMB)]
        wv = [pp.tile([128, 256], F32R, tag=f"wv{k}", name=f"wv{k}") for k in range(N_DIMB)]
        wo = [pp.tile([128, DIM], F32R, tag=f"wo{d}", name=f"wo{d}") for d in range(2)]
        # q/k head-dim-major: partition block hp holds heads (2hp, 2hp+1)
        qt = [
            [pp.tile([128, QT], F32R, tag=f"qt{i}_{c}", name=f"qt{i}_{c}") for c in range(N_QT)]
            for i in range(2)
        ]
        kt = [
            [pp.tile([128, QT], F32R, tag=f"kt{i}_{c}", name=f"kt{i}_{c}") for c in range(N_QT)]
            for i in range(2)
        ]
        # v (token-major) + ones column, per key block: [128, 4 heads, 65]
        vh = [
            pp.tile([128, H_PER_CORE, 2 * HD], F32R, tag=f"vh{b}", name=f"vh{b}")
            for b in range(N_KB)
        ]
        # ones row for the K=1 denominator-broadcast matmuls (row 64 used)
        ones = pp.tile([128, 64], F32R, tag="ones", name="ones")
        nc.vector.memset(ones[:].bitcast(F32), 1.0)

        # ---- input DMAs: kb-major so the kb=0..7 chains fill in order; the
        # xT columns arrive chunk-by-chunk so tile 0's projections start early
        for kb in range(N_DIMB):
            nc.sync.dma_start(wqk[kb][:], wqkT[kb * 128 : (kb + 1) * 128, :])
            nc.sync.dma_start(xt[kb][0][:], xT[kb * 128 : (kb + 1) * 128, 0:QT])
        for kb in range(N_DIMB):
            nc.sync.dma_start(wv[kb][:], wvT[kb * 128 : (kb + 1) * 128, :])
        for ct in range(1, N_QT):
            cs = slice(ct * QT, (ct + 1) * QT)
            for kb in range(N_DIMB):
                nc.sync.dma_start(xt[kb][ct][:], xT[kb * 128 : (kb + 1) * 128, cs])
        for db in range(2):
            nc.sync.dma_start(wo[db][:], woT[db * 128 : (db + 1) * 128, :])

        def qk_chain(tt, ob):  # ob 0,1 -> q pair blocks; 2,3 -> k pair blocks
            ps = mmps.tile([128, 512], F32, tag="mm512", name=f"qk_ps{tt}_{ob}")
            for kb in range(N_DIMB):
                nc.tensor.matmul(
                    ps[:],
                    wqk[kb][:, ob * 128 : (ob + 1) * 128],
                    xt[kb][tt][:],
                    start=(kb == 0),
                    stop=(kb == N_DIMB - 1),
                )
            dest = (qt if ob < 2 else kt)[ob % 2][tt]
            nc.vector.tensor_copy(dest[:], ps[:])

        def v_chain(tb):
            ps = mmps.tile([128, 256], F32, tag="mm512", name=f"v_ps{tb}")
            for kb in range(N_DIMB):
                nc.tensor.matmul(
                    ps[:],
                    xt[kb][tb // 4][:, (tb % 4) * KB : (tb % 4 + 1) * KB],
                    wv[kb][:],
                    start=(kb == 0),
                    stop=(kb == N_DIMB - 1),
                )
            nc.vector.tensor_copy(vh[tb][:, :, 0:HD], ps[:])
            nc.vector.memset(vh[tb][:, :, HD : 2 * HD].bitcast(F32), 1.0)

        def proj_pieces(tt):
            for ob in range(4):
                yield lambda ob=ob: qk_chain(tt, ob)
            for tb in range(4 * tt, 4 * tt + 4):
                yield lambda tb=tb: v_chain(tb)

        def y_pieces_split(tt, os_pair):
            """Output projection split per head-pair half: the os_pair[0]
            halves (a) can run as fillers inside attend(tt) right after pair
            0's normalize; the os_pair[1] halves (b) accumulate via DVE adds
            once pair 1 lands."""
            ysbs = {}

            def get_ysb(t4):
                if t4 not in ysbs:
                    ysbs[t4] = ypool.tile(
                        [128, DIM], F32, tag="ysb", name=f"ysb{tt}_{t4}"
                    )
                return ysbs[t4]

            a_pieces, b_pieces = [], []
            for t4 in range(4):
                trows = slice(t4 * 128, (t4 + 1) * 128)
                for jb in range(2):

                    def pa(t4=t4, jb=jb, trows=trows):
                        yps = mmps.tile(
                            [128, 512], F32, tag="mm512", name=f"ya{tt}_{t4}_{jb}"
                        )
                        nc.tensor.matmul(
                            yps[:],
                            os_pair[0][:, trows],
                            wo[0][:, jb * 512 : (jb + 1) * 512],
                            start=True,
                            stop=True,
                        )
                        nc.vector.tensor_copy(
                            get_ysb(t4)[:, jb * 512 : (jb + 1) * 512], yps[:]
                        )

                    def pb(t4=t4, jb=jb, trows=trows):
                        yps = mmps.tile(
                            [128, 512], F32, tag="mm512", name=f"yb{tt}_{t4}_{jb}"
                        )
                        nc.tensor.matmul(
                            yps[:],
                            os_pair[1][:, trows],
                            wo[1][:, jb * 512 : (jb + 1) * 512],
                            start=True,
                            stop=True,
                        )
                        ysb = get_ysb(t4)
                        dest = ysb[:, jb * 512 : (jb + 1) * 512]
                        nc.vector.tensor_add(dest, dest, yps[:])
                        if jb == 1:
                            nc.sync.dma_start(
                                y[tt * QT + t4 * 128 : tt * QT + (t4 + 1) * 128, :],
                                ysb[:],
                            )

                    a_pieces.append(pa)
                    b_pieces.append(pb)
            return a_pieces, b_pieces

        def attend(tt, os_pair, fillers, late=None):
            nb = 4 * (tt + 1)  # allowed key blocks for this query tile
            n_steps = 2 * nb
            step = 0
            done_fill = 0
            n_fill = len(fillers)
            late_q = deque()
            late_done = 0
            late_start = None

            def fill():
                nonlocal done_fill, late_done
                want = (step + 1) * n_fill // n_steps
                while done_fill < want and fillers:
                    fillers.popleft()()
                    done_fill += 1
                if late_start is not None and late_q:
                    lsteps = max(n_steps - late_start, 1)
                    lwant = (step - late_start + 1) * len_late // lsteps
                    while late_done < lwant and late_q:
                        late_q.popleft()()
                        late_done += 1

            for hp in range(2):  # head pair (2hp, 2hp+1)
                ot = [
                    otps.tile([128, QT], F32, tag="ot", name=f"ot{tt}_{hp}_{i}")
                    for i in range(2)
                ]

                def s_mm(b):
                    """S^T for key block b, both heads, into one 2-bank tile.
                    Also allocates the exp target and emits its mask memset
                    here, a block early, so the memset sits ahead of any
                    filler work in the in-order DVE stream."""
                    diag = b - 4 * tt
                    d = diag * 128 if diag >= 0 else 0
                    s = sps.tile([128, 2 * QT], F32, tag="s2", name=f"s{tt}_{hp}_{b}")
                    for i in range(2):
                        rows = slice(i * 64, i * 64 + 64)
                        nc.tensor.matmul(
                            s[:, i * QT + d : (i + 1) * QT],
                            kt[hp][b // 4][rows, (b % 4) * KB : (b % 4 + 1) * KB],
                            qt[hp][tt][rows, d:QT],
                            start=True,
                            stop=True,
                        )
                    p = ppool.tile([128, 2 * QT], F32R, tag="p", name=f"p{tt}_{hp}_{b}")
                    return s, p

                s_tiles = {0: s_mm(0)}
                for b in range(nb):
                    if b + 1 < nb:
                        s_tiles[b + 1] = s_mm(b + 1)
                    diag = b - 4 * tt
                    d = diag * 128 if diag >= 0 else 0
                    s, p = s_tiles.pop(b)
                    if diag < 0:
                        nc.scalar.activation(p[:], s[:], EXP, scale=SCALE)
                    else:
                        # one exp for both heads over cols >= d (all rows),
                        # then zero the masked corner (rows 64-127 of each
                        # head attend only cols >= d+64) AFTER the exp
                        s2 = s[:].rearrange("p (h c) -> p h c", h=2)
                        p2 = p[:].rearrange("p (h c) -> p h c", h=2)
                        nc.scalar.activation(
                            p2[:, :, d:QT], s2[:, :, d:QT], EXP, scale=SCALE
                        )
                        nc.vector.memset(
                            p2[64:128, :, d : d + 64].bitcast(F32), 0.0
                        )
                    for i in range(2):
                        nc.tensor.matmul(
                            ot[i][:, d:QT],
                            vh[b][:, 2 * hp + i, :],
                            p[:, i * QT + d : (i + 1) * QT],
                            start=(b == 0),
                            stop=(b == nb - 1),
                        )
                    fill()
                    step += 1

                # normalize: os_pair[hp][i*64:(i+1)*64] = ot[i][0:64] / ot[i][64]
                for i in range(2):
                    # denominator already replicated on partitions 64-127 by
                    # the 64 ones-columns in vhat; partition-shifted DVE
                    # reciprocal brings 1/denom to partitions 0-63
                    rb = rpool.tile([64, QT], F32, tag="rb", name=f"rb{tt}_{hp}_{i}")
                    nc.vector.reciprocal(rb[:], ot[i][64:128, :])
                    if i == 0:
                        # head at partitions 0-63: write os_pair directly
                        nc.vector.tensor_mul(
                            os_pair[hp][0:64, :], ot[i][0:64, :], rb[:]
                        )
                    else:
                        # head at partitions 64-127: partition-shifted DVE copy
                        sc = scpool.tile(
                            [64, QT], F32R, tag="sc", name=f"sc{tt}_{hp}_{i}"
                        )
                        nc.vector.tensor_mul(sc[:], ot[i][0:64, :], rb[:])
                        nc.vector.tensor_copy(os_pair[hp][64:128, :], sc[:])

                if hp == 0 and late:
                    late_q.extend(late)
                    late_start = step
                    len_late = len(late)

            while fillers:
                fillers.popleft()()
            while late_q:
                late_q.popleft()()

        def y_pieces_paired(tt, os_pair):
            pieces = []
            for t4 in range(4):
                trows = slice(t4 * 128, (t4 + 1) * 128)
                ysb = ypool.tile([128, DIM], F32, tag="ysb", name=f"ysb{tt}_{t4}")
                for jb in range(2):

                    def piece(t4=t4, jb=jb, ysb=ysb, trows=trows):
                        yps = mmps.tile(
                            [128, 512], F32, tag="mm512", name=f"y_ps{tt}_{t4}_{jb}"
                        )
                        for db in range(2):
                            nc.tensor.matmul(
                                yps[:],
                                os_pair[db][:, trows],
                                wo[db][:, jb * 512 : (jb + 1) * 512],
                                start=(db == 0),
                                stop=(db == 1),
                            )
                        nc.vector.tensor_copy(ysb[:, jb * 512 : (jb + 1) * 512], yps[:])
                        if jb == 1:
                            nc.sync.dma_start(
                                y[tt * QT + t4 * 128 : tt * QT + (t4 + 1) * 128, :],
                                ysb[:],
                            )

                    pieces.append(piece)
            return pieces

        # ---- the pipeline ----
        for piece in proj_pieces(0):
            piece()
        prev_b = None
        for tt in range(N_QT):
            os_pair = [
                ospool.tile([128, QT], F32R, tag=f"os{i}", name=f"os{i}_{tt}")
                for i in range(2)
            ]
            fillers = deque()
            a = deque(proj_pieces(tt + 1)) if tt + 1 < N_QT else deque()
            b = deque(prev_b) if prev_b is not None else deque()
            while a or b:
                if b:
                    fillers.append(b.popleft())
                if a:
                    fillers.append(a.popleft())
            split = SPLIT_Y and tt == N_QT - 1
            if split:
                a_pieces, b_pieces = y_pieces_split(tt, os_pair)
                attend(tt, os_pair, fillers, late=a_pieces)
                prev_b = b_pieces
            else:
                attend(tt, os_pair, fillers)
                prev_b = y_pieces_paired(tt, os_pair)
        for piece in prev_b:
            piece()


def build():
    global _CACHED_NC
    if _CACHED_NC is not None:
        return _CACHED_NC
    nc = bacc.Bacc(
        "TRN2", target_bir_lowering=False, debug=False, enable_asserts=False
    )
    xT = nc.dram_tensor("xT", [DIM, T], F32R, kind="ExternalInput").ap()
    wqkT = nc.dram_tensor("wqkT", [DIM, 512], F32R, kind="ExternalInput").ap()
    wvT = nc.dram_tensor("wvT", [DIM, 256], F32R, kind="ExternalInput").ap()
    woT = nc.dram_tensor("woutT", [256, DIM], F32R, kind="ExternalInput").ap()
    y = nc.dram_tensor("y", [T, DIM], F32, kind="ExternalOutput").ap()
    with tile.TileContext(nc) as tc:
        _emit(nc, tc, xT, wqkT, wvT, woT, y)
    nc.compile()
    _CACHED_NC = nc
    return nc


def make_in_maps(x, Wqkv, Wout):
    """Host-side sharding: core c = (batch c//4, head-group c%4)."""
    in_maps = []
    for c in range(8):
        b, hg = divmod(c, 4)
        hs = hg * H_PER_CORE
        r0, r1 = hs * HD, (hs + H_PER_CORE) * HD
        qrows = Wqkv[r0:r1]
        krows = Wqkv[DIM + r0 : DIM + r1]
        vrows = Wqkv[2 * DIM + r0 : 2 * DIM + r1]
        in_maps.append(
            {
                "xT": np.ascontiguousarray(x[b].T),
                "wqkT": np.ascontiguousarray(np.concatenate([qrows, krows], 0).T),
                "wvT": np.ascontiguousarray(vrows.T),
                "woutT": np.ascontiguousarray(Wout[:, r0:r1].T),
            }
        )
    return in_maps


def kernel(x, Wqkv, Wout):
    x = np.asarray(x, dtype=np.float32)
    Wqkv = np.asarray(Wqkv, dtype=np.float32)
    Wout = np.asarray(Wout, dtype=np.float32)
    nc = build()
    in_maps = make_in_maps(x, Wqkv, Wout)
    res = run_bass_kernel_spmd(nc, in_maps, core_ids=list(range(8)))
    out = np.zeros((B, T, DIM), np.float32)
    for c in range(8):
        out[c // 4] += res.results[c]["y"]
    return out



# revision 12
# speedup vs baseline: 1.1218x; 1.1218x over previous
"""Trainium2 Bass kernel for block-causal (chunked) multi-head attention.

Computes, for x:[2,2048,1024], Wqkv:[3072,1024], Wout:[1024,1024]:
    qkv = x @ Wqkv.T ; per-head scaled scores; block-causal mask
    (causal OR same 64-chunk == full attention to all chunks <= own chunk);
    softmax; out = attn @ v ; y = out @ Wout.T

Sharding over 8 NeuronCores: data-parallel over batch (2) x tensor-parallel
over heads (16 heads -> 4 per core).  Each core projects q/k/v for its 4
heads, runs attention, and computes a partial output projection against its
256 columns of Wout; the host sums the 4 partials per batch element.

All SBUF operands are float16 (PE runs f16 at 1 row/cycle with no small-tile
penalty, DMA bytes halve, and the ~1e-3 quantization error is far inside the
tolerance); PSUM accumulation stays f32.

On-chip layout avoids all transposes: the host hands each core
  xT     [1024, 2048]  (x[b] transposed, f16)
  wqkT   [1024, 512]   (Wqkv rows for its 4 heads' q,k -> transposed)
  wvT    [1024, 256]   (v rows transposed)
  woutT  [256, 1024]   (Wout columns for its head-slice, transposed)
Scores are computed transposed (S^T[tk, tq]) so that the attention matmul
P^T -> (attn @ V) needs no transposes, and the softmax denominator comes
for free from a ones-column appended to V.  The block-causal mask is
realized structurally: masked-out key blocks are simply never computed, and
the diagonal blocks use rectangular sub-views (chunk granularity 64).

Engines execute their instruction streams in order, so the emission is a
software pipeline over the 4 query tiles: the TensorE stream for the
(ScalarE-paced) attention of tile t is interleaved with "filler" matmul
chains -- the output projection of tile t-1, the q projections of tile t+1,
and (deadline-scheduled) tile t's OWN k/v projections, which are only
consumed from key-block 4t onward -- keeping the PE busy through every exp
dependency stall.
"""

import sys

if "/opt/trn_rl_repo" not in sys.path:
    sys.path.insert(0, "/opt/trn_rl_repo")

from collections import deque

import numpy as np

import concourse.bass as bass  # noqa: F401  (registers types)
import concourse.mybir as mybir
import concourse.tile as tile
from concourse import bacc
from concourse.bass_utils import run_bass_kernel_spmd

F32 = mybir.dt.float32
F16 = mybir.dt.float16
EXP = mybir.ActivationFunctionType.Exp

B = 2
T = 2048
DIM = 1024
N_HEADS = 16
HD = 64
CHUNK = 64
H_PER_CORE = 4  # 16 heads / (8 cores / 2 batches)
QT = 512  # query tile (free dim of S^T matmuls)
KB = 128  # key block (contraction block of AV matmuls)
N_QT = T // QT  # 4
N_KB = T // KB  # 16
N_DIMB = DIM // 128  # 8 contraction blocks for the projections
SCALE = 1.0 / np.sqrt(HD)

_CACHED_NC = None


def _emit(nc, tc, xT, wqkT, wvT, woT, y):
    po = tc.tile_pool  # shorthand

    with (
        po(name="persist", bufs=1) as pp,
        po(name="s_ps", bufs=2, space="PSUM") as sps,  # [128,1024] score slots
        po(name="mm_ps", bufs=2, space="PSUM") as mmps,  # [128,512] proj/y slots
        po(name="ot_ps", bufs=2, space="PSUM") as otps,  # [128,512] outT slots
        po(name="pbuf", bufs=4) as ppool,  # exp(S^T) tiles
        po(name="osbuf", bufs=2) as ospool,  # assembled normalized outT
        po(name="rbuf", bufs=2) as rpool,  # reciprocal denominators
        po(name="ybuf", bufs=3) as ypool,
    ):
        # ---- persistent SBUF tensors (kb stacked in the free dim so input
        # DMAs batch into a few large transfers) ----
        xt = [pp.tile([128, N_DIMB, QT], F16, tag=f"xt{c}", name=f"xt{c}") for c in range(N_QT)]
        wqk = pp.tile([128, N_DIMB, 512], F16, tag="wqk", name="wqk")
        wv = pp.tile([128, N_DIMB, 256], F16, tag="wv", name="wv")
        wo = pp.tile([128, 2, DIM], F16, tag="wo", name="wo")
        # q/k head-dim-major: partition block hp holds heads (2hp, 2hp+1)
        qt = [
            [pp.tile([128, QT], F16, tag=f"qt{i}_{c}", name=f"qt{i}_{c}") for c in range(N_QT)]
            for i in range(2)
        ]
        kt = [
            [pp.tile([128, QT], F16, tag=f"kt{i}_{c}", name=f"kt{i}_{c}") for c in range(N_QT)]
            for i in range(2)
        ]
        # v (token-major) + ones column, per key block: [128, 4 heads, 65]
        vh = [
            pp.tile([128, H_PER_CORE, 2 * HD], F16, tag=f"vh{b}", name=f"vh{b}")
            for b in range(N_KB)
        ]

        # ---- input DMAs: tile-0 inputs arrive in fine chunks so the first
        # projection chains start early; the rest are single batched DMAs
        def src3(t, rows, cols):  # [rows*128, cols] -> [128, rows-chunks, cols]
            return t.rearrange("(k p) n -> p k n", p=128)

        chunks = [(0, 1), (1, 2), (2, 4), (4, 6), (6, 8)]
        for k0, k1 in chunks:  # fine pacing for wqk + xt tile 0
            nc.sync.dma_start(
                wqk[:, k0:k1, :], src3(wqkT[128 * k0 : 128 * k1, :], k1 - k0, 512)
            )
            nc.sync.dma_start(
                xt[0][:, k0:k1, :], src3(xT[128 * k0 : 128 * k1, 0:QT], k1 - k0, QT)
            )
        nc.sync.dma_start(wv[:], src3(wvT, N_DIMB, 256))
        for ct in range(1, N_QT):
            nc.sync.dma_start(
                xt[ct][:], src3(xT[:, ct * QT : (ct + 1) * QT], N_DIMB, QT)
            )
        nc.sync.dma_start(wo[:], src3(woT, 2, DIM))

        def qk_chain(tt, ob):  # ob 0,1 -> q pair blocks; 2,3 -> k pair blocks
            ps = mmps.tile([128, 512], F32, tag="mm512", name=f"qk_ps{tt}_{ob}")
            for kb in range(N_DIMB):
                nc.tensor.matmul(
                    ps[:],
                    wqk[:, kb, ob * 128 : (ob + 1) * 128],
                    xt[tt][:, kb, :],
                    start=(kb == 0),
                    stop=(kb == N_DIMB - 1),
                )
            dest = (qt if ob < 2 else kt)[ob % 2][tt]
            nc.vector.tensor_copy(dest[:], ps[:])

        def v_chain(tb):
            ps = mmps.tile([128, 256], F32, tag="mm512", name=f"v_ps{tb}")
            for kb in range(N_DIMB):
                nc.tensor.matmul(
                    ps[:],
                    xt[tb // 4][:, kb, (tb % 4) * KB : (tb % 4 + 1) * KB],
                    wv[:, kb, :],
                    start=(kb == 0),
                    stop=(kb == N_DIMB - 1),
                )
            nc.vector.tensor_copy(vh[tb][:, :, 0:HD], ps[:])
            nc.vector.memset(vh[tb][:, :, HD : 2 * HD], 1.0)

        def q_pieces(tt):  # q projections: needed before attend(tt) starts
            for ob in range(2):
                yield None, (lambda ob=ob: qk_chain(tt, ob))

        def kv_pieces(tt):
            """k/v projections of tile tt, with deadlines (hp, key-block)
            inside attend(tt) itself: k for head-pair hp is first consumed by
            the S matmul of block 4*tt of that pair; v[b] by the AV matmul of
            block b of pair 0."""
            yield (0, 4 * tt), (lambda: qk_chain(tt, 2))
            for tb in range(4 * tt, 4 * tt + 4):
                yield (0, tb), (lambda tb=tb: v_chain(tb))
            yield (1, 4 * tt), (lambda: qk_chain(tt, 3))

        def attend(tt, os_pair, fillers, reserve=0):
            nb = 4 * (tt + 1)  # allowed key blocks for this query tile
            step = 0
            done_fill = 0
            n_fill = len(fillers) - reserve
            # pace fillers by cumulative exp cost (the ScalarE is the pacing
            # engine through an attend), not by step count
            w = []
            for hp in range(2):
                for bb in range(nb):
                    diag = bb - 4 * tt
                    d = diag * 128 if diag >= 0 else 0
                    w.append(2 * (QT - d) * 0.833 + 200.0)
            cumw = np.cumsum(w) / sum(w)

            def run_piece():
                nonlocal done_fill
                fillers.popleft()[1]()
                done_fill += 1

            def fill():
                want = min(int(round(n_fill * cumw[step])), n_fill)
                while done_fill < want and fillers:
                    run_piece()

            def flush(hp, b):
                # force-run any deadline piece due at or before (hp, b)
                nonlocal done_fill
                if not any(k is not None and k <= (hp, b) for k, _ in fillers):
                    return
                rest = deque()
                while fillers:
                    k, fn = fillers.popleft()
                    if k is not None and k <= (hp, b):
                        fn()
                        done_fill += 1
                    else:
                        rest.append((k, fn))
                fillers.extend(rest)

            for hp in range(2):  # head pair (2hp, 2hp+1)
                ot = [
                    otps.tile([128, QT], F32, tag="ot", name=f"ot{tt}_{hp}_{i}")
                    for i in range(2)
                ]

                def s_mm(b):
                    """S^T for key block b, both heads, into one 2-bank tile."""
                    diag = b - 4 * tt
                    d = diag * 128 if diag >= 0 else 0
                    s = sps.tile([128, 2 * QT], F32, tag="s2", name=f"s{tt}_{hp}_{b}")
                    for i in range(2):
                        rows = slice(i * 64, i * 64 + 64)
                        nc.tensor.matmul(
                            s[:, i * QT + d : (i + 1) * QT],
                            kt[hp][b // 4][rows, (b % 4) * KB : (b % 4 + 1) * KB],
                            qt[hp][tt][rows, d:QT],
                            start=True,
                            stop=True,
                        )
                    p = ppool.tile([128, 2 * QT], F16, tag="p", name=f"p{tt}_{hp}_{b}")
                    return s, p

                flush(hp, 0)
                s_tiles = {0: s_mm(0)}
                for b in range(nb):
                    if b + 1 < nb:
                        flush(hp, b + 1)
                        s_tiles[b + 1] = s_mm(b + 1)
                    diag = b - 4 * tt
                    d = diag * 128 if diag >= 0 else 0
                    s, p = s_tiles.pop(b)
                    if diag < 0:
                        nc.scalar.activation(p[:], s[:], EXP, scale=SCALE)
                    else:
                        # one exp for both heads over cols >= d (all rows),
                        # then zero the masked corner (rows 64-127 of each
                        # head attend only cols >= d+64) AFTER the exp
                        s2 = s[:].rearrange("p (h c) -> p h c", h=2)
                        p2 = p[:].rearrange("p (h c) -> p h c", h=2)
                        nc.scalar.activation(
                            p2[:, :, d:QT], s2[:, :, d:QT], EXP, scale=SCALE
                        )
                        nc.vector.memset(p2[64:128, :, d : d + 64], 0.0)
                    for i in range(2):
                        nc.tensor.matmul(
                            ot[i][:, d:QT],
                            vh[b][:, 2 * hp + i, :],
                            p[:, i * QT + d : (i + 1) * QT],
                            start=(b == 0),
                            stop=(b == nb - 1),
                        )
                    fill()
                    step += 1

                # normalize: os_pair[hp][i*64:(i+1)*64] = ot[i][0:64] / ot[i][64]
                # (denominator replicated on partitions 64-127 by the 64
                # ones-columns in vh; DVE ops partition-shift as needed).
                # The last normalize is split into column halves so the first
                # y pieces can start before the whole chain drains.
                halves = [(0, QT)] if not (tt == N_QT - 1 and hp == 1) else [
                    (0, QT // 2),
                    (QT // 2, QT),
                ]
                for c0, c1 in halves:
                    for i in range(2):
                        rb = rpool.tile(
                            [64, c1 - c0],
                            F32,
                            tag=f"rb{c1 - c0}",
                            name=f"rb{tt}_{hp}_{i}_{c0}",
                        )
                        nc.vector.reciprocal(rb[:], ot[i][64:128, c0:c1])
                        nc.vector.tensor_mul(
                            os_pair[hp][i * 64 : (i + 1) * 64, c0:c1],
                            ot[i][0:64, c0:c1],
                            rb[:],
                        )

            while fillers:
                run_piece()

        def y_pieces_paired(tt, os_pair, tail=False):
            pieces = []
            for t4 in range(4):
                trows = slice(t4 * 128, (t4 + 1) * 128)
                ysb = ypool.tile([128, DIM], F16, tag="ysb", name=f"ysb{tt}_{t4}")
                for jb in range(2):

                    def piece(t4=t4, jb=jb, ysb=ysb, trows=trows):
                        yps = mmps.tile(
                            [128, 512], F32, tag="mm512", name=f"y_ps{tt}_{t4}_{jb}"
                        )
                        for db in range(2):
                            nc.tensor.matmul(
                                yps[:],
                                os_pair[db][:, trows],
                                wo[:, db, jb * 512 : (jb + 1) * 512],
                                start=(db == 0),
                                stop=(db == 1),
                            )
                        dest = ysb[:, jb * 512 : (jb + 1) * 512]
                        rows = slice(tt * QT + t4 * 128, tt * QT + (t4 + 1) * 128)
                        if tail:
                            # drain phase: alternate the PSUM->SBUF copies
                            # between the (idle) ScalarE and the DVE, and DMA
                            # each half out as soon as it is assembled, so the
                            # final copy+DMA chain is as short as possible
                            if (t4 + jb) % 2 == 0:
                                nc.scalar.copy(dest, yps[:])
                            else:
                                nc.vector.tensor_copy(dest, yps[:])
                            nc.sync.dma_start(
                                y[rows, jb * 512 : (jb + 1) * 512], dest
                            )
                        else:
                            nc.vector.tensor_copy(dest, yps[:])
                            if jb == 1:
                                nc.sync.dma_start(y[rows, :], ysb[:])

                    pieces.append((None, piece))
            return pieces

        # ---- the pipeline ----
        # Tile 0's attention needs only q/k of head-pair 0 and v[0] to start;
        # the rest of its projection streams in as deadline fillers.  Later
        # tiles pre-run only their q chains.
        qk_chain(0, 0)
        qk_chain(0, 2)
        v_chain(0)
        prev_b = None
        for tt in range(N_QT):
            os_pair = [
                ospool.tile([128, QT], F16, tag=f"os{i}", name=f"os{i}_{tt}")
                for i in range(2)
            ]
            nxt = deque()
            if tt + 1 < N_QT:
                nxt.extend(q_pieces(tt + 1))
            if tt:
                late = deque(kv_pieces(tt))
            else:
                late = deque(
                    [((0, tb), (lambda tb=tb: v_chain(tb))) for tb in (1, 2, 3)]
                    + [((1, 0), (lambda: qk_chain(0, 1))),
                       ((1, 0), (lambda: qk_chain(0, 3)))]
                )
            b = deque(prev_b) if prev_b is not None else deque()
            fillers = deque()
            while late or b or nxt:
                if late:
                    fillers.append(late.popleft())
                if b:
                    fillers.append(b.popleft())
                if late:
                    fillers.append(late.popleft())
                if nxt:
                    fillers.append(nxt.popleft())
            attend(tt, os_pair, fillers)
            prev_b = y_pieces_paired(tt, os_pair, tail=(tt == N_QT - 1))
        for _, piece in prev_b:
            piece()


def build():
    global _CACHED_NC
    if _CACHED_NC is not None:
        return _CACHED_NC
    nc = bacc.Bacc(
        "TRN2", target_bir_lowering=False, debug=False, enable_asserts=False
    )
    xT = nc.dram_tensor("xT", [DIM, T], F16, kind="ExternalInput").ap()
    wqkT = nc.dram_tensor("wqkT", [DIM, 512], F16, kind="ExternalInput").ap()
    wvT = nc.dram_tensor("wvT", [DIM, 256], F16, kind="ExternalInput").ap()
    woT = nc.dram_tensor("woutT", [256, DIM], F16, kind="ExternalInput").ap()
    y = nc.dram_tensor("y", [T, DIM], F16, kind="ExternalOutput").ap()
    with tile.TileContext(nc) as tc:
        _emit(nc, tc, xT, wqkT, wvT, woT, y)
    nc.compile()
    _CACHED_NC = nc
    return nc


def make_in_maps(x, Wqkv, Wout):
    """Host-side sharding: core c = (batch c//4, head-group c%4)."""
    in_maps = []
    for c in range(8):
        b, hg = divmod(c, 4)
        hs = hg * H_PER_CORE
        r0, r1 = hs * HD, (hs + H_PER_CORE) * HD
        qrows = Wqkv[r0:r1]
        krows = Wqkv[DIM + r0 : DIM + r1]
        vrows = Wqkv[2 * DIM + r0 : 2 * DIM + r1]
        in_maps.append(
            {
                "xT": np.ascontiguousarray(x[b].T.astype(np.float16)),
                "wqkT": np.ascontiguousarray(
                    np.concatenate([qrows, krows], 0).T.astype(np.float16)
                ),
                "wvT": np.ascontiguousarray(vrows.T.astype(np.float16)),
                "woutT": np.ascontiguousarray(Wout[:, r0:r1].T.astype(np.float16)),
            }
        )
    return in_maps


def kernel(x, Wqkv, Wout):
    x = np.asarray(x, dtype=np.float32)
    Wqkv = np.asarray(Wqkv, dtype=np.float32)
    Wout = np.asarray(Wout, dtype=np.float32)
    nc = build()
    in_maps = make_in_maps(x, Wqkv, Wout)
    res = run_bass_kernel_spmd(nc, in_maps, core_ids=list(range(8)))
    out = np.zeros((B, T, DIM), np.float32)
    for c in range(8):
        out[c // 4] += res.results[c]["y"].astype(np.float32)
    return out


# revision 47
# speedup vs baseline: 1.1622x; 1.0361x over previous
"""Trainium2 Bass kernel for block-causal (chunked) multi-head attention.

Computes, for x:[2,2048,1024], Wqkv:[3072,1024], Wout:[1024,1024]:
    qkv = x @ Wqkv.T ; per-head scaled scores; block-causal mask
    (causal OR same 64-chunk == full attention to all chunks <= own chunk);
    softmax; out = attn @ v ; y = out @ Wout.T

Sharding over 8 NeuronCores: data-parallel over batch (2) x tensor-parallel
over heads (16 heads -> 4 per core).  Each core projects q/k/v for its 4
heads, runs attention, and computes a partial output projection against its
256 columns of Wout; the host sums the 4 partials per batch element.

All SBUF operands are float16 (PE runs f16 at 1 row/cycle with no small-tile
penalty, DMA bytes halve, and the ~1e-3 quantization error is far inside the
tolerance); PSUM accumulation stays f32.

On-chip layout avoids all transposes: the host hands each core
  xT     [1024, 2048]  (x[b] transposed, f16)
  wqkT   [1024, 512]   (Wqkv rows for its 4 heads' q,k -> transposed)
  wvT    [1024, 256]   (v rows transposed)
  woutT  [256, 1024]   (Wout columns for its head-slice, transposed)
Scores are computed transposed (S^T[tk, tq]) so that the attention matmul
P^T -> (attn @ V) needs no transposes, and the softmax denominator comes
for free from a ones-column appended to V.  The block-causal mask is
realized structurally: masked-out key blocks are simply never computed, and
the diagonal blocks use rectangular sub-views (chunk granularity 64).

Engines execute their instruction streams in order, so the emission is a
software pipeline over the 4 query tiles: the TensorE stream for the
(ScalarE-paced) attention of tile t is interleaved with "filler" matmul
chains -- the output projection of tile t-1, the q projections of tile t+1,
and (deadline-scheduled) tile t's OWN k/v projections, which are only
consumed from key-block 4t onward -- keeping the PE busy through every exp
dependency stall.
"""

import sys

if "/opt/trn_rl_repo" not in sys.path:
    sys.path.insert(0, "/opt/trn_rl_repo")

from collections import deque

import numpy as np

import concourse.bass as bass  # noqa: F401  (registers types)
import concourse.mybir as mybir
import concourse.tile as tile
from concourse import bacc
from concourse.bass_utils import run_bass_kernel_spmd

F32 = mybir.dt.float32
F16 = mybir.dt.float16
EXP = mybir.ActivationFunctionType.Exp

B = 2
T = 2048
DIM = 1024
N_HEADS = 16
HD = 64
CHUNK = 64
H_PER_CORE = 4  # 16 heads / (8 cores / 2 batches)
QT = 512  # query tile (free dim of S^T matmuls)
KB = 128  # key block (contraction block of AV matmuls)
N_QT = T // QT  # 4
N_KB = T // KB  # 16
N_DIMB = DIM // 128  # 8 contraction blocks for the projections
SCALE = 1.0 / np.sqrt(HD)

_CACHED_NC = None


def _emit(nc, tc, xT, wqkT, wvT, woT, y):
    po = tc.tile_pool  # shorthand

    with (
        po(name="persist", bufs=1) as pp,
        po(name="s_ps", bufs=2, space="PSUM") as sps,  # [128,1024] score slots
        po(name="mm_ps", bufs=2, space="PSUM") as mmps,  # [128,512] proj/y slots
        po(name="ot_ps", bufs=2, space="PSUM") as otps,  # [128,512] outT slots
        po(name="pbuf", bufs=4) as ppool,  # exp(S^T) tiles
        po(name="osbuf", bufs=2) as ospool,  # assembled normalized outT
        po(name="rbuf", bufs=2) as rpool,  # reciprocal denominators
        po(name="ybuf", bufs=6) as ypool,
    ):
        # ---- persistent SBUF tensors (kb stacked in the free dim so input
        # DMAs batch into a few large transfers) ----
        xt = [pp.tile([128, N_DIMB, QT], F16, tag=f"xt{c}", name=f"xt{c}") for c in range(N_QT)]
        wqk = pp.tile([128, N_DIMB, 512], F16, tag="wqk", name="wqk")
        wv = pp.tile([128, N_DIMB, 256], F16, tag="wv", name="wv")
        wo = pp.tile([128, 2, DIM], F16, tag="wo", name="wo")
        # q/k head-dim-major: partition block hp holds heads (2hp, 2hp+1)
        qt = [
            [pp.tile([128, QT], F16, tag=f"qt{i}_{c}", name=f"qt{i}_{c}") for c in range(N_QT)]
            for i in range(2)
        ]
        kt = [
            [pp.tile([128, QT], F16, tag=f"kt{i}_{c}", name=f"kt{i}_{c}") for c in range(N_QT)]
            for i in range(2)
        ]
        # v (token-major) + ones column, per key block: [128, 4 heads, 65]
        vh = [
            pp.tile([128, H_PER_CORE, 2 * HD], F16, tag=f"vh{b}", name=f"vh{b}")
            for b in range(N_KB)
        ]

        # ---- input DMAs: tile-0 inputs arrive in fine chunks so the first
        # projection chains start early; the rest are single batched DMAs
        def src3(t, rows, cols):  # [rows*128, cols] -> [128, rows-chunks, cols]
            return t.rearrange("(k p) n -> p k n", p=128)

        chunks = [(0, 1), (1, 2), (2, 4), (4, 6), (6, 8)]
        for k0, k1 in chunks:  # fine pacing for wqk + xt tile 0
            nc.sync.dma_start(
                wqk[:, k0:k1, :], src3(wqkT[128 * k0 : 128 * k1, :], k1 - k0, 512)
            )
            nc.sync.dma_start(
                xt[0][:, k0:k1, :], src3(xT[128 * k0 : 128 * k1, 0:QT], k1 - k0, QT)
            )
        nc.sync.dma_start(wv[:], src3(wvT, N_DIMB, 256))
        for half in range(2):  # xt tile 1 halved: its q chains fill attend(0)
            ks = slice(4 * half, 4 * half + 4)
            nc.sync.dma_start(
                xt[1][:, ks, :],
                src3(xT[512 * half : 512 * half + 512, QT : 2 * QT], 4, QT),
            )
        for ct in range(2, N_QT):
            nc.sync.dma_start(
                xt[ct][:], src3(xT[:, ct * QT : (ct + 1) * QT], N_DIMB, QT)
            )
        nc.sync.dma_start(wo[:], src3(woT, 2, DIM))

        # the ones-columns of every vh tile have no input dependency:
        # emit them all at t=0 while the DVE is otherwise idle
        for tb in range(N_KB):
            nc.vector.memset(vh[tb][:, :, HD : 2 * HD], 1.0)

        def qk_chain(tt, ob):  # ob 0,1 -> q pair blocks; 2,3 -> k pair blocks
            ps = mmps.tile([128, 512], F32, tag="mm512", name=f"qk_ps{tt}_{ob}")
            for kb in range(N_DIMB):
                nc.tensor.matmul(
                    ps[:],
                    wqk[:, kb, ob * 128 : (ob + 1) * 128],
                    xt[tt][:, kb, :],
                    start=(kb == 0),
                    stop=(kb == N_DIMB - 1),
                )
            dest = (qt if ob < 2 else kt)[ob % 2][tt]
            nc.vector.tensor_copy(dest[:], ps[:])

        def v_chain(tb):
            ps = mmps.tile([128, 256], F32, tag="mm512", name=f"v_ps{tb}")
            for kb in range(N_DIMB):
                nc.tensor.matmul(
                    ps[:],
                    xt[tb // 4][:, kb, (tb % 4) * KB : (tb % 4 + 1) * KB],
                    wv[:, kb, :],
                    start=(kb == 0),
                    stop=(kb == N_DIMB - 1),
                )
            nc.vector.tensor_copy(vh[tb][:, :, 0:HD], ps[:])

        def q_pieces(tt):  # q projections: needed before attend(tt) starts
            for ob in range(2):
                yield None, (lambda ob=ob: qk_chain(tt, ob))

        def kv_pieces(tt):
            """k/v projections of tile tt, with deadlines (hp, key-block)
            inside attend(tt) itself: k for head-pair hp is first consumed by
            the S matmul of block 4*tt of that pair; v[b] by the AV matmul of
            block b of pair 0."""
            yield (0, 4 * tt), (lambda: qk_chain(tt, 2))
            for tb in range(4 * tt, 4 * tt + 4):
                yield (0, tb), (lambda tb=tb: v_chain(tb))
            yield (1, 4 * tt), (lambda: qk_chain(tt, 3))

        def attend(tt, os_pair, fillers, deferred, reserve=0):
            nb = 4 * (tt + 1)  # allowed key blocks for this query tile
            step = 0
            done_fill = 0
            n_fill = len(fillers) - reserve
            # pace fillers by cumulative exp cost (the ScalarE is the pacing
            # engine through an attend), not by step count
            w = []
            for hp in range(2):
                for bb in range(nb):
                    diag = bb - 4 * tt
                    d = diag * 128 if diag >= 0 else 0
                    exp_ns = 2 * (QT - d) * 0.833 + 200.0
                    pe_ns = 4 * (QT - d) * 0.4167
                    w.append(max(exp_ns - pe_ns, 60.0))
            cumw = np.cumsum(w) / sum(w)

            def run_piece():
                nonlocal done_fill
                fillers.popleft()[1]()
                done_fill += 1

            def fill():
                want = min((step + 1) * n_fill // (2 * nb), n_fill)
                while done_fill < want and fillers:
                    run_piece()

            def flush(hp, b):
                # force-run any deadline piece due at or before (hp, b)
                nonlocal done_fill
                if not any(k is not None and k <= (hp, b) for k, _ in fillers):
                    return
                rest = deque()
                while fillers:
                    k, fn = fillers.popleft()
                    if k is not None and k <= (hp, b):
                        fn()
                        done_fill += 1
                    else:
                        rest.append((k, fn))
                fillers.extend(rest)

            for hp in range(2):  # head pair (2hp, 2hp+1)
                ot = [
                    otps.tile([128, QT], F32, tag="ot", name=f"ot{tt}_{hp}_{i}")
                    for i in range(2)
                ]

                # normalize: os_pair[hp][i*64:(i+1)*64] = ot[i][0:64]/ot[i][64]
                # (denominator replicated on partitions 64-127 by the 64
                # ones-columns in vh; DVE ops partition-shift as needed)
                def norm(c0, c1, hp=hp, ot=ot):
                    for i in range(2):
                        rb = rpool.tile(
                            [64, c1 - c0],
                            F32,
                            tag=f"rb{c1 - c0}",
                            name=f"rb{tt}_{hp}_{i}_{c0}",
                        )
                        nc.vector.reciprocal(rb[:], ot[i][64:128, c0:c1])
                        nc.vector.tensor_mul(
                            os_pair[hp][i * 64 : (i + 1) * 64, c0:c1],
                            ot[i][0:64, c0:c1],
                            rb[:],
                        )

                norm_mid = lambda: norm(0, QT // 2)

                def s_mm(b):
                    """S^T for key block b, both heads, into one 2-bank tile."""
                    diag = b - 4 * tt
                    d = diag * 128 if diag >= 0 else 0
                    s = sps.tile([128, 2 * QT], F32, tag="s2", name=f"s{tt}_{hp}_{b}")
                    for i in range(2):
                        rows = slice(i * 64, i * 64 + 64)
                        nc.tensor.matmul(
                            s[:, i * QT + d : (i + 1) * QT],
                            kt[hp][b // 4][rows, (b % 4) * KB : (b % 4 + 1) * KB],
                            qt[hp][tt][rows, d:QT],
                            start=True,
                            stop=True,
                        )
                    p = ppool.tile([128, 2 * QT], F16, tag="p", name=f"p{tt}_{hp}_{b}")
                    return s, p

                flush(hp, 0)
                s_tiles = {0: s_mm(0)}
                for b in range(nb):
                    if b + 1 < nb:
                        flush(hp, b + 1)
                        s_tiles[b + 1] = s_mm(b + 1)
                    diag = b - 4 * tt
                    d = diag * 128 if diag >= 0 else 0
                    s, p = s_tiles.pop(b)
                    if diag < 0:
                        nc.scalar.activation(p[:], s[:], EXP, scale=SCALE)
                    else:
                        # one exp for both heads over cols >= d (all rows),
                        # then zero the masked corner (rows 64-127 of each
                        # head attend only cols >= d+64) AFTER the exp
                        s2 = s[:].rearrange("p (h c) -> p h c", h=2)
                        p2 = p[:].rearrange("p (h c) -> p h c", h=2)
                        nc.scalar.activation(
                            p2[:, :, d:QT], s2[:, :, d:QT], EXP, scale=SCALE
                        )
                        # keep the corner memset off the DVE stream for the
                        # last two blocks of the final pair, where it must
                        # not queue behind the staggered normalize
                        if tt == N_QT - 1 and hp == 1 and b >= nb - 2:
                            nc.gpsimd.memset(p2[64:128, :, d : d + 64], 0.0)
                        else:
                            nc.vector.memset(p2[64:128, :, d : d + 64], 0.0)
                    for i in range(2):
                        nc.tensor.matmul(
                            ot[i][:, d:QT],
                            vh[b][:, 2 * hp + i, :],
                            p[:, i * QT + d : (i + 1) * QT],
                            start=(b == 0),
                            stop=(b == nb - 1),
                        )
                    fill()
                    step += 1
                    if tt == N_QT - 1 and hp == 1 and b == nb - 3:
                        # columns [0:256) of ot are final (blocks 14,15 only
                        # touch cols >= 256): normalize them now, overlapping
                        # the last two attention steps
                        norm_mid()

                # the last normalize is staggered: cols [0:256) were
                # emitted mid-loop, cols [256:512) are deferred into the
                # drain so the first y pieces' DVE copies aren't queued
                # behind the whole chain
                if tt == N_QT - 1 and hp == 1:
                    deferred.append(lambda: norm(QT // 2, QT))
                else:
                    norm(0, QT)

            while fillers:
                run_piece()

        def y_pieces_paired(tt, os_pair, tail=False):
            pieces = []
            # during the drain the score PSUM banks are free the moment the
            # last exp has read them: two 2-bank score tiles host four of the
            # eight pieces, so no piece's matmuls ever wait on a prior copy
            ys2 = (
                [sps.tile([128, 2 * QT], F32, tag="s2", name=f"ys2_{j}") for j in range(2)]
                if tail
                else None
            )
            for t4 in range(4):
                trows = slice(t4 * 128, (t4 + 1) * 128)
                ysb = ypool.tile([128, DIM], F16, tag="ysb", name=f"ysb{tt}_{t4}")
                for jb in range(2):

                    def piece(t4=t4, jb=jb, ysb=ysb, trows=trows):
                        k = t4 * 2 + jb
                        s2slot = {1: (0, 0), 2: (0, 1), 4: (1, 0), 5: (1, 1)}
                        if tail and k in s2slot:
                            j, h = s2slot[k]
                            yps = ys2[j][:, h * 512 : (h + 1) * 512]
                        else:
                            yps = mmps.tile(
                                [128, 512], F32, tag="mm512", name=f"y_ps{tt}_{t4}_{jb}"
                            )
                        for db in range(2):
                            nc.tensor.matmul(
                                yps[:],
                                os_pair[db][:, trows],
                                wo[:, db, jb * 512 : (jb + 1) * 512],
                                start=(db == 0),
                                stop=(db == 1),
                            )
                        dest = ysb[:, jb * 512 : (jb + 1) * 512]
                        rows = slice(tt * QT + t4 * 128, tt * QT + (t4 + 1) * 128)
                        if tail:
                            # drain phase: alternate the PSUM->SBUF copies
                            # between the (idle) ScalarE and the DVE so the
                            # final copy chain halves
                            if (t4 + jb) % 2 == 0:
                                nc.scalar.copy(dest, yps[:])
                            else:
                                nc.vector.tensor_copy(dest, yps[:])
                        else:
                            nc.vector.tensor_copy(dest, yps[:])
                        if jb == 1:
                            nc.sync.dma_start(y[rows, :], ysb[:])

                    pieces.append((None, piece))
            return pieces

        # ---- the pipeline ----
        # Tile 0's q/k chains run kb-interleaved across four concurrent PSUM
        # accumulators (the attention pools are untouched this early), so the
        # in-order PE stream is never blocked behind one chain's wait for the
        # next input chunk.
        pre_ps = [
            mmps.tile([128, 512], F32, tag="mm512", name=f"pre_ps{j}")
            for j in range(2)
        ] + [
            otps.tile([128, QT], F32, tag="ot", name=f"pre_ps{j + 2}")
            for j in range(2)
        ]
        pre_obs = [0, 2, 1, 3]
        for kb in range(N_DIMB):
            for j, ob in enumerate(pre_obs):
                nc.tensor.matmul(
                    pre_ps[j][:],
                    wqk[:, kb, ob * 128 : (ob + 1) * 128],
                    xt[0][:, kb, :],
                    start=(kb == 0),
                    stop=(kb == N_DIMB - 1),
                )
        for j, ob in enumerate(pre_obs):
            dest = (qt if ob < 2 else kt)[ob % 2][0]
            nc.vector.tensor_copy(dest[:], pre_ps[j][:])
        v_chain(0)
        prev_b = None
        for tt in range(N_QT):
            os_pair = [
                ospool.tile([128, QT], F16, tag=f"os{i}", name=f"os{i}_{tt}")
                for i in range(2)
            ]
            nxt = deque()
            if tt + 1 < N_QT:
                nxt.extend(q_pieces(tt + 1))
            if tt:
                late = deque(kv_pieces(tt))
            else:
                late = deque(
                    [((0, tb), (lambda tb=tb: v_chain(tb))) for tb in (1, 2, 3)]
                )
            b = deque(prev_b) if prev_b is not None else deque()
            fillers = deque()
            while late or b or nxt:
                if late:
                    fillers.append(late.popleft())
                if b:
                    fillers.append(b.popleft())
                if late:
                    fillers.append(late.popleft())
                if nxt:
                    fillers.append(nxt.popleft())
            deferred = []
            attend(tt, os_pair, fillers, deferred)
            prev_b = y_pieces_paired(tt, os_pair, tail=(tt == N_QT - 1))
        for idx, (_, piece) in enumerate(prev_b):
            if idx == 3:
                for d in deferred:
                    d()
            piece()


def build():
    global _CACHED_NC
    if _CACHED_NC is not None:
        return _CACHED_NC
    nc = bacc.Bacc(
        "TRN2", target_bir_lowering=False, debug=False, enable_asserts=False
    )
    xT = nc.dram_tensor("xT", [DIM, T], F16, kind="ExternalInput").ap()
    wqkT = nc.dram_tensor("wqkT", [DIM, 512], F16, kind="ExternalInput").ap()
    wvT = nc.dram_tensor("wvT", [DIM, 256], F16, kind="ExternalInput").ap()
    woT = nc.dram_tensor("woutT", [256, DIM], F16, kind="ExternalInput").ap()
    y = nc.dram_tensor("y", [T, DIM], F16, kind="ExternalOutput").ap()
    with tile.TileContext(nc) as tc:
        _emit(nc, tc, xT, wqkT, wvT, woT, y)
    nc.compile()
    _CACHED_NC = nc
    return nc


def make_in_maps(x, Wqkv, Wout):
    """Host-side sharding: core c = (batch c//4, head-group c%4)."""
    in_maps = []
    for c in range(8):
        b, hg = divmod(c, 4)
        hs = hg * H_PER_CORE
        r0, r1 = hs * HD, (hs + H_PER_CORE) * HD
        qrows = Wqkv[r0:r1]
        krows = Wqkv[DIM + r0 : DIM + r1]
        vrows = Wqkv[2 * DIM + r0 : 2 * DIM + r1]
        in_maps.append(
            {
                "xT": np.ascontiguousarray(x[b].T.astype(np.float16)),
                "wqkT": np.ascontiguousarray(
                    np.concatenate([qrows, krows], 0).T.astype(np.float16)
                ),
                "wvT": np.ascontiguousarray(vrows.T.astype(np.float16)),
                "woutT": np.ascontiguousarray(Wout[:, r0:r1].T.astype(np.float16)),
            }
        )
    return in_maps


def kernel(x, Wqkv, Wout):
    x = np.asarray(x, dtype=np.float32)
    Wqkv = np.asarray(Wqkv, dtype=np.float32)
    Wout = np.asarray(Wout, dtype=np.float32)
    nc = build()
    in_maps = make_in_maps(x, Wqkv, Wout)
    res = run_bass_kernel_spmd(nc, in_maps, core_ids=list(range(8)))
    out = np.zeros((B, T, DIM), np.float32)
    for c in range(8):
        out[c // 4] += res.results[c]["y"].astype(np.float32)
    return out


# revision 57
# speedup vs baseline: 1.1682x; 1.0051x over previous
"""Trainium2 Bass kernel for block-causal (chunked) multi-head attention.

Computes, for x:[2,2048,1024], Wqkv:[3072,1024], Wout:[1024,1024]:
    qkv = x @ Wqkv.T ; per-head scaled scores; block-causal mask
    (causal OR same 64-chunk == full attention to all chunks <= own chunk);
    softmax; out = attn @ v ; y = out @ Wout.T

Sharding over 8 NeuronCores: data-parallel over batch (2) x tensor-parallel
over heads (16 heads -> 4 per core).  Each core projects q/k/v for its 4
heads, runs attention, and computes a partial output projection against its
256 columns of Wout; the host sums the 4 partials per batch element.

All SBUF operands are float16 (PE runs f16 at 1 row/cycle with no small-tile
penalty, DMA bytes halve, and the ~1e-3 quantization error is far inside the
tolerance); PSUM accumulation stays f32.

On-chip layout avoids all transposes: the host hands each core
  xT     [1024, 2048]  (x[b] transposed, f16)
  wqkT   [1024, 512]   (Wqkv rows for its 4 heads' q,k -> transposed)
  wvT    [1024, 256]   (v rows transposed)
  woutT  [256, 1024]   (Wout columns for its head-slice, transposed)
Scores are computed transposed (S^T[tk, tq]) so that the attention matmul
P^T -> (attn @ V) needs no transposes, and the softmax denominator comes
for free from a ones-column appended to V.  The block-causal mask is
realized structurally: masked-out key blocks are simply never computed, and
the diagonal blocks use rectangular sub-views (chunk granularity 64).

Engines execute their instruction streams in order, so the emission is a
software pipeline over the 4 query tiles: the TensorE stream for the
(ScalarE-paced) attention of tile t is interleaved with "filler" matmul
chains -- the output projection of tile t-1, the q projections of tile t+1,
and (deadline-scheduled) tile t's OWN k/v projections, which are only
consumed from key-block 4t onward -- keeping the PE busy through every exp
dependency stall.
"""

import sys

if "/opt/trn_rl_repo" not in sys.path:
    sys.path.insert(0, "/opt/trn_rl_repo")

from collections import deque

import numpy as np

import concourse.bass as bass  # noqa: F401  (registers types)
import concourse.mybir as mybir
import concourse.tile as tile
from concourse import bacc
from concourse.bass_utils import run_bass_kernel_spmd

F32 = mybir.dt.float32
F16 = mybir.dt.float16
EXP = mybir.ActivationFunctionType.Exp

B = 2
T = 2048
DIM = 1024
N_HEADS = 16
HD = 64
CHUNK = 64
H_PER_CORE = 4  # 16 heads / (8 cores / 2 batches)
QT = 512  # query tile (free dim of S^T matmuls)
KB = 128  # key block (contraction block of AV matmuls)
N_QT = T // QT  # 4
N_KB = T // KB  # 16
N_DIMB = DIM // 128  # 8 contraction blocks for the projections
SCALE = 1.0 / np.sqrt(HD)

_CACHED_NC = None


def _emit(nc, tc, xT, wqkT, wvT, woT, y):
    po = tc.tile_pool  # shorthand

    with (
        po(name="persist", bufs=1) as pp,
        po(name="s_ps", bufs=2, space="PSUM") as sps,  # [128,1024] score slots
        po(name="mm_ps", bufs=2, space="PSUM") as mmps,  # [128,512] proj/y slots
        po(name="ot_ps", bufs=2, space="PSUM") as otps,  # [128,512] outT slots
        po(name="pbuf", bufs=6) as ppool,  # exp(S^T) tiles
        po(name="osbuf", bufs=2) as ospool,  # assembled normalized outT
        po(name="rbuf", bufs=2) as rpool,  # reciprocal denominators
        po(name="ybuf", bufs=6) as ypool,
    ):
        # ---- persistent SBUF tensors (kb stacked in the free dim so input
        # DMAs batch into a few large transfers) ----
        xt = [pp.tile([128, N_DIMB, QT], F16, tag=f"xt{c}", name=f"xt{c}") for c in range(N_QT)]
        wqk = pp.tile([128, N_DIMB, 512], F16, tag="wqk", name="wqk")
        wv = pp.tile([128, N_DIMB, 256], F16, tag="wv", name="wv")
        wo = pp.tile([128, 2, DIM], F16, tag="wo", name="wo")
        # q/k head-dim-major: partition block hp holds heads (2hp, 2hp+1)
        qt = [
            [pp.tile([128, QT], F16, tag=f"qt{i}_{c}", name=f"qt{i}_{c}") for c in range(N_QT)]
            for i in range(2)
        ]
        kt = [
            [pp.tile([128, QT], F16, tag=f"kt{i}_{c}", name=f"kt{i}_{c}") for c in range(N_QT)]
            for i in range(2)
        ]
        # v (token-major) + ones column, per key block: [128, 4 heads, 65]
        vh = [
            pp.tile([128, H_PER_CORE, 2 * HD], F16, tag=f"vh{b}", name=f"vh{b}")
            for b in range(N_KB)
        ]

        # ---- input DMAs: tile-0 inputs arrive in fine chunks so the first
        # projection chains start early; the rest are single batched DMAs
        def src3(t, rows, cols):  # [rows*128, cols] -> [128, rows-chunks, cols]
            return t.rearrange("(k p) n -> p k n", p=128)

        chunks = [(0, 1), (1, 2), (2, 4), (4, 6), (6, 8)]
        for k0, k1 in chunks:  # fine pacing for wqk + xt tile 0
            nc.sync.dma_start(
                wqk[:, k0:k1, :], src3(wqkT[128 * k0 : 128 * k1, :], k1 - k0, 512)
            )
            nc.sync.dma_start(
                xt[0][:, k0:k1, :], src3(xT[128 * k0 : 128 * k1, 0:QT], k1 - k0, QT)
            )
        nc.sync.dma_start(wv[:], src3(wvT, N_DIMB, 256))
        for half in range(2):  # xt tile 1 halved: its q chains fill attend(0)
            ks = slice(4 * half, 4 * half + 4)
            nc.sync.dma_start(
                xt[1][:, ks, :],
                src3(xT[512 * half : 512 * half + 512, QT : 2 * QT], 4, QT),
            )
        for ct in range(2, N_QT):
            nc.sync.dma_start(
                xt[ct][:], src3(xT[:, ct * QT : (ct + 1) * QT], N_DIMB, QT)
            )
        nc.sync.dma_start(wo[:], src3(woT, 2, DIM))

        # the ones-columns of every vh tile have no input dependency:
        # emit them all at t=0 while the DVE is otherwise idle
        for tb in range(N_KB):
            nc.vector.memset(vh[tb][:, :, HD : 2 * HD], 1.0)

        def qk_chain(tt, ob):  # ob 0,1 -> q pair blocks; 2,3 -> k pair blocks
            ps = mmps.tile([128, 512], F32, tag="mm512", name=f"qk_ps{tt}_{ob}")
            for kb in range(N_DIMB):
                nc.tensor.matmul(
                    ps[:],
                    wqk[:, kb, ob * 128 : (ob + 1) * 128],
                    xt[tt][:, kb, :],
                    start=(kb == 0),
                    stop=(kb == N_DIMB - 1),
                )
            dest = (qt if ob < 2 else kt)[ob % 2][tt]
            nc.vector.tensor_copy(dest[:], ps[:])

        def v_chain(tb):
            ps = mmps.tile([128, 256], F32, tag="mm512", name=f"v_ps{tb}")
            for kb in range(N_DIMB):
                nc.tensor.matmul(
                    ps[:],
                    xt[tb // 4][:, kb, (tb % 4) * KB : (tb % 4 + 1) * KB],
                    wv[:, kb, :],
                    start=(kb == 0),
                    stop=(kb == N_DIMB - 1),
                )
            nc.vector.tensor_copy(vh[tb][:, :, 0:HD], ps[:])

        def q_pieces(tt):  # q projections: needed before attend(tt) starts
            for ob in range(2):
                yield None, (lambda ob=ob: qk_chain(tt, ob))

        def kv_pieces(tt):
            """k/v projections of tile tt, with deadlines (hp, key-block)
            inside attend(tt) itself: k for head-pair hp is first consumed by
            the S matmul of block 4*tt of that pair; v[b] by the AV matmul of
            block b of pair 0."""
            yield (0, 4 * tt), (lambda: qk_chain(tt, 2))
            for tb in range(4 * tt, 4 * tt + 4):
                yield (0, tb), (lambda tb=tb: v_chain(tb))
            yield (1, 4 * tt), (lambda: qk_chain(tt, 3))

        def attend(tt, os_pair, fillers, deferred, reserve=0):
            nb = 4 * (tt + 1)  # allowed key blocks for this query tile
            step = 0
            done_fill = 0
            n_fill = len(fillers) - reserve
            # pace fillers by cumulative exp cost (the ScalarE is the pacing
            # engine through an attend), not by step count
            w = []
            for hp in range(2):
                for bb in range(nb):
                    diag = bb - 4 * tt
                    d = diag * 128 if diag >= 0 else 0
                    exp_ns = 2 * (QT - d) * 0.833 + 200.0
                    pe_ns = 4 * (QT - d) * 0.4167
                    w.append(max(exp_ns - pe_ns, 60.0))
            cumw = np.cumsum(w) / sum(w)

            def run_piece():
                nonlocal done_fill
                fillers.popleft()[1]()
                done_fill += 1

            def fill():
                want = min((step + 1) * n_fill // (2 * nb), n_fill)
                while done_fill < want and fillers:
                    run_piece()

            def flush(hp, b):
                # force-run any deadline piece due at or before (hp, b)
                nonlocal done_fill
                if not any(k is not None and k <= (hp, b) for k, _ in fillers):
                    return
                rest = deque()
                while fillers:
                    k, fn = fillers.popleft()
                    if k is not None and k <= (hp, b):
                        fn()
                        done_fill += 1
                    else:
                        rest.append((k, fn))
                fillers.extend(rest)

            for hp in range(2):  # head pair (2hp, 2hp+1)
                ot = [
                    otps.tile([128, QT], F32, tag="ot", name=f"ot{tt}_{hp}_{i}")
                    for i in range(2)
                ]

                # normalize: os_pair[hp][i*64:(i+1)*64] = ot[i][0:64]/ot[i][64]
                # (denominator replicated on partitions 64-127 by the 64
                # ones-columns in vh; DVE ops partition-shift as needed)
                def norm(c0, c1, hp=hp, ot=ot):
                    for i in range(2):
                        rb = rpool.tile(
                            [64, c1 - c0],
                            F32,
                            tag=f"rb{c1 - c0}",
                            name=f"rb{tt}_{hp}_{i}_{c0}",
                        )
                        nc.vector.reciprocal(rb[:], ot[i][64:128, c0:c1])
                        nc.vector.tensor_mul(
                            os_pair[hp][i * 64 : (i + 1) * 64, c0:c1],
                            ot[i][0:64, c0:c1],
                            rb[:],
                        )

                norm_mid = lambda: norm(0, QT // 2)

                def s_mm(b):
                    """S^T for key block b, both heads, into one 2-bank tile."""
                    diag = b - 4 * tt
                    d = diag * 128 if diag >= 0 else 0
                    s = sps.tile([128, 2 * QT], F32, tag="s2", name=f"s{tt}_{hp}_{b}")
                    for i in range(2):
                        rows = slice(i * 64, i * 64 + 64)
                        nc.tensor.matmul(
                            s[:, i * QT + d : (i + 1) * QT],
                            kt[hp][b // 4][rows, (b % 4) * KB : (b % 4 + 1) * KB],
                            qt[hp][tt][rows, d:QT],
                            start=True,
                            stop=True,
                        )
                    p = ppool.tile([128, 2 * QT], F16, tag="p", name=f"p{tt}_{hp}_{b}")
                    return s, p

                flush(hp, 0)
                s_tiles = {0: s_mm(0)}
                for b in range(nb):
                    if b + 1 < nb:
                        flush(hp, b + 1)
                        s_tiles[b + 1] = s_mm(b + 1)
                    diag = b - 4 * tt
                    d = diag * 128 if diag >= 0 else 0
                    s, p = s_tiles.pop(b)
                    if diag < 0:
                        nc.scalar.activation(p[:], s[:], EXP, scale=SCALE)
                    else:
                        # one exp for both heads over cols >= d (all rows),
                        # then zero the masked corner (rows 64-127 of each
                        # head attend only cols >= d+64) AFTER the exp
                        s2 = s[:].rearrange("p (h c) -> p h c", h=2)
                        p2 = p[:].rearrange("p (h c) -> p h c", h=2)
                        nc.scalar.activation(
                            p2[:, :, d:QT], s2[:, :, d:QT], EXP, scale=SCALE
                        )
                        # keep the corner memset off the DVE stream for the
                        # last two blocks of the final pair, where it must
                        # not queue behind the staggered normalize
                        if tt == N_QT - 1 and hp == 1 and b >= nb - 2:
                            nc.gpsimd.memset(p2[64:128, :, d : d + 64], 0.0)
                        else:
                            nc.vector.memset(p2[64:128, :, d : d + 64], 0.0)
                    for i in range(2):
                        nc.tensor.matmul(
                            ot[i][:, d:QT],
                            vh[b][:, 2 * hp + i, :],
                            p[:, i * QT + d : (i + 1) * QT],
                            start=(b == 0),
                            stop=(b == nb - 1),
                        )
                    fill()
                    step += 1
                    if tt == N_QT - 1 and hp == 1 and b == nb - 3:
                        # columns [0:256) of ot are final (blocks 14,15 only
                        # touch cols >= 256): normalize them now, overlapping
                        # the last two attention steps
                        norm_mid()

                # the last normalize is staggered: cols [0:256) were
                # emitted mid-loop, cols [256:512) are deferred into the
                # drain so the first y pieces' DVE copies aren't queued
                # behind the whole chain
                if tt == N_QT - 1 and hp == 1:
                    deferred.append(lambda: norm(QT // 2, QT))
                else:
                    norm(0, QT)

            while fillers:
                run_piece()

        def y_pieces_paired(tt, os_pair, tail=False):
            pieces = []
            # during the drain the score PSUM banks are free the moment the
            # last exp has read them: two 2-bank score tiles host four of the
            # eight pieces, so no piece's matmuls ever wait on a prior copy
            ys2 = (
                [sps.tile([128, 2 * QT], F32, tag="s2", name=f"ys2_{j}") for j in range(2)]
                if tail
                else None
            )
            for t4 in range(4):
                trows = slice(t4 * 128, (t4 + 1) * 128)
                ysb = ypool.tile([128, DIM], F16, tag="ysb", name=f"ysb{tt}_{t4}")
                for jb in range(2):

                    def piece(t4=t4, jb=jb, ysb=ysb, trows=trows):
                        k = t4 * 2 + jb
                        s2slot = {1: (0, 0), 2: (0, 1), 4: (1, 0), 5: (1, 1)}
                        if tail and k in s2slot:
                            j, h = s2slot[k]
                            yps = ys2[j][:, h * 512 : (h + 1) * 512]
                        else:
                            yps = mmps.tile(
                                [128, 512], F32, tag="mm512", name=f"y_ps{tt}_{t4}_{jb}"
                            )
                        for db in range(2):
                            nc.tensor.matmul(
                                yps[:],
                                os_pair[db][:, trows],
                                wo[:, db, jb * 512 : (jb + 1) * 512],
                                start=(db == 0),
                                stop=(db == 1),
                            )
                        dest = ysb[:, jb * 512 : (jb + 1) * 512]
                        rows = slice(tt * QT + t4 * 128, tt * QT + (t4 + 1) * 128)
                        if tail:
                            # drain phase: alternate the PSUM->SBUF copies
                            # between the (idle) ScalarE and the DVE so the
                            # final copy chain halves
                            if (t4 + jb) % 2 == 0:
                                nc.scalar.copy(dest, yps[:])
                            else:
                                nc.vector.tensor_copy(dest, yps[:])
                        else:
                            nc.vector.tensor_copy(dest, yps[:])
                        if jb == 1:
                            nc.sync.dma_start(y[rows, :], ysb[:])

                    pieces.append((None, piece))
            return pieces

        # ---- the pipeline ----
        # Tile 0's q/k chains run kb-interleaved across four concurrent PSUM
        # accumulators (the attention pools are untouched this early), so the
        # in-order PE stream is never blocked behind one chain's wait for the
        # next input chunk.
        pre_ps = [
            mmps.tile([128, 512], F32, tag="mm512", name=f"pre_ps{j}")
            for j in range(2)
        ] + [
            otps.tile([128, QT], F32, tag="ot", name=f"pre_ps{j + 2}")
            for j in range(2)
        ]
        pre_obs = [0, 2, 1, 3]
        for kb in range(N_DIMB):
            for j, ob in enumerate(pre_obs):
                nc.tensor.matmul(
                    pre_ps[j][:],
                    wqk[:, kb, ob * 128 : (ob + 1) * 128],
                    xt[0][:, kb, :],
                    start=(kb == 0),
                    stop=(kb == N_DIMB - 1),
                )
        for j, ob in enumerate(pre_obs):
            dest = (qt if ob < 2 else kt)[ob % 2][0]
            nc.vector.tensor_copy(dest[:], pre_ps[j][:])
        v_chain(0)
        prev_b = None
        for tt in range(N_QT):
            os_pair = [
                ospool.tile([128, QT], F16, tag=f"os{i}", name=f"os{i}_{tt}")
                for i in range(2)
            ]
            nxt = deque()
            if tt + 1 < N_QT:
                nxt.extend(q_pieces(tt + 1))
            if tt:
                late = deque(kv_pieces(tt))
            else:
                late = deque(
                    [((0, tb), (lambda tb=tb: v_chain(tb))) for tb in (1, 2, 3)]
                )
            b = deque(prev_b) if prev_b is not None else deque()
            fillers = deque()
            while late or b or nxt:
                if late:
                    fillers.append(late.popleft())
                if b:
                    fillers.append(b.popleft())
                if late:
                    fillers.append(late.popleft())
                if nxt:
                    fillers.append(nxt.popleft())
            deferred = []
            attend(tt, os_pair, fillers, deferred)
            prev_b = y_pieces_paired(tt, os_pair, tail=(tt == N_QT - 1))
        for idx, (_, piece) in enumerate(prev_b):
            if idx == 4:
                for d in deferred:
                    d()
            piece()


def build():
    global _CACHED_NC
    if _CACHED_NC is not None:
        return _CACHED_NC
    nc = bacc.Bacc(
        "TRN2", target_bir_lowering=False, debug=False, enable_asserts=False
    )
    xT = nc.dram_tensor("xT", [DIM, T], F16, kind="ExternalInput").ap()
    wqkT = nc.dram_tensor("wqkT", [DIM, 512], F16, kind="ExternalInput").ap()
    wvT = nc.dram_tensor("wvT", [DIM, 256], F16, kind="ExternalInput").ap()
    woT = nc.dram_tensor("woutT", [256, DIM], F16, kind="ExternalInput").ap()
    y = nc.dram_tensor("y", [T, DIM], F16, kind="ExternalOutput").ap()
    with tile.TileContext(nc) as tc:
        _emit(nc, tc, xT, wqkT, wvT, woT, y)
    nc.compile()
    _CACHED_NC = nc
    return nc


def make_in_maps(x, Wqkv, Wout):
    """Host-side sharding: core c = (batch c//4, head-group c%4)."""
    in_maps = []
    for c in range(8):
        b, hg = divmod(c, 4)
        hs = hg * H_PER_CORE
        r0, r1 = hs * HD, (hs + H_PER_CORE) * HD
        qrows = Wqkv[r0:r1]
        krows = Wqkv[DIM + r0 : DIM + r1]
        vrows = Wqkv[2 * DIM + r0 : 2 * DIM + r1]
        in_maps.append(
            {
                "xT": np.ascontiguousarray(x[b].T.astype(np.float16)),
                "wqkT": np.ascontiguousarray(
                    np.concatenate([qrows, krows], 0).T.astype(np.float16)
                ),
                "wvT": np.ascontiguousarray(vrows.T.astype(np.float16)),
                "woutT": np.ascontiguousarray(Wout[:, r0:r1].T.astype(np.float16)),
            }
        )
    return in_maps


def kernel(x, Wqkv, Wout):
    x = np.asarray(x, dtype=np.float32)
    Wqkv = np.asarray(Wqkv, dtype=np.float32)
    Wout = np.asarray(Wout, dtype=np.float32)
    nc = build()
    in_maps = make_in_maps(x, Wqkv, Wout)
    res = run_bass_kernel_spmd(nc, in_maps, core_ids=list(range(8)))
    out = np.zeros((B, T, DIM), np.float32)
    for c in range(8):
        out[c // 4] += res.results[c]["y"].astype(np.float32)
    return out


# revision 72
# speedup vs baseline: 1.1732x; 1.0043x over previous
"""Trainium2 Bass kernel for block-causal (chunked) multi-head attention.

Computes, for x:[2,2048,1024], Wqkv:[3072,1024], Wout:[1024,1024]:
    qkv = x @ Wqkv.T ; per-head scaled scores; block-causal mask
    (causal OR same 64-chunk == full attention to all chunks <= own chunk);
    softmax; out = attn @ v ; y = out @ Wout.T

Sharding over 8 NeuronCores: data-parallel over batch (2) x tensor-parallel
over heads (16 heads -> 4 per core).  Each core projects q/k/v for its 4
heads, runs attention, and computes a partial output projection against its
256 columns of Wout; the host sums the 4 partials per batch element.

All SBUF operands are float16 (PE runs f16 at 1 row/cycle with no small-tile
penalty, DMA bytes halve, and the ~1e-3 quantization error is far inside the
tolerance); PSUM accumulation stays f32.

On-chip layout avoids all transposes: the host hands each core
  xT     [1024, 2048]  (x[b] transposed, f16)
  wqkT   [1024, 512]   (Wqkv rows for its 4 heads' q,k -> transposed)
  wvT    [1024, 256]   (v rows transposed)
  woutT  [256, 1024]   (Wout columns for its head-slice, transposed)
Scores are computed transposed (S^T[tk, tq]) so that the attention matmul
P^T -> (attn @ V) needs no transposes, and the softmax denominator comes
for free from a ones-column appended to V.  The block-causal mask is
realized structurally: masked-out key blocks are simply never computed, and
the diagonal blocks use rectangular sub-views (chunk granularity 64).

Engines execute their instruction streams in order, so the emission is a
software pipeline over the 4 query tiles: the TensorE stream for the
(ScalarE-paced) attention of tile t is interleaved with "filler" matmul
chains -- the output projection of tile t-1, the q projections of tile t+1,
and (deadline-scheduled) tile t's OWN k/v projections, which are only
consumed from key-block 4t onward -- keeping the PE busy through every exp
dependency stall.
"""

import sys

if "/opt/trn_rl_repo" not in sys.path:
    sys.path.insert(0, "/opt/trn_rl_repo")

from collections import deque

import numpy as np

import concourse.bass as bass  # noqa: F401  (registers types)
import concourse.mybir as mybir
import concourse.tile as tile
from concourse import bacc
from concourse.bass_utils import run_bass_kernel_spmd

F32 = mybir.dt.float32
F16 = mybir.dt.float16
EXP = mybir.ActivationFunctionType.Exp

B = 2
T = 2048
DIM = 1024
N_HEADS = 16
HD = 64
CHUNK = 64
H_PER_CORE = 4  # 16 heads / (8 cores / 2 batches)
QT = 512  # query tile (free dim of S^T matmuls)
KB = 128  # key block (contraction block of AV matmuls)
N_QT = T // QT  # 4
N_KB = T // KB  # 16
N_DIMB = DIM // 128  # 8 contraction blocks for the projections
SCALE = 1.0 / np.sqrt(HD)

_CACHED_NC = None


def _emit(nc, tc, xT, wqkT, wvT, woT, y):
    po = tc.tile_pool  # shorthand

    with (
        po(name="persist", bufs=1) as pp,
        po(name="s_ps", bufs=2, space="PSUM") as sps,  # [128,1024] score slots
        po(name="mm_ps", bufs=2, space="PSUM") as mmps,  # [128,512] proj/y slots
        po(name="ot_ps", bufs=2, space="PSUM") as otps,  # [128,512] outT slots
        po(name="pbuf", bufs=6) as ppool,  # exp(S^T) tiles
        po(name="osbuf", bufs=2) as ospool,  # assembled normalized outT
        po(name="rbuf", bufs=2) as rpool,  # reciprocal denominators
        po(name="ybuf", bufs=6) as ypool,
    ):
        # ---- persistent SBUF tensors (kb stacked in the free dim so input
        # DMAs batch into a few large transfers) ----
        xt = [pp.tile([128, N_DIMB, QT], F16, tag=f"xt{c}", name=f"xt{c}") for c in range(N_QT)]
        wqk = pp.tile([128, N_DIMB, 512], F16, tag="wqk", name="wqk")
        wv = pp.tile([128, N_DIMB, 256], F16, tag="wv", name="wv")
        wo = pp.tile([128, 2, DIM], F16, tag="wo", name="wo")
        # q/k head-dim-major: partition block hp holds heads (2hp, 2hp+1)
        qt = [
            [pp.tile([128, QT], F16, tag=f"qt{i}_{c}", name=f"qt{i}_{c}") for c in range(N_QT)]
            for i in range(2)
        ]
        kt = [
            [pp.tile([128, QT], F16, tag=f"kt{i}_{c}", name=f"kt{i}_{c}") for c in range(N_QT)]
            for i in range(2)
        ]
        # v (token-major) + ones columns, per key block: [128, 4 heads, 2*64]
        vh = [
            pp.tile([128, H_PER_CORE, 2 * HD], F16, tag=f"vh{b}", name=f"vh{b}")
            for b in range(N_KB)
        ]

        # ---- input DMAs: tile-0 inputs arrive in fine chunks so the first
        # projection chains start early; the rest are single batched DMAs
        def src3(t, rows, cols):  # [rows*128, cols] -> [128, rows-chunks, cols]
            return t.rearrange("(k p) n -> p k n", p=128)

        chunks = [(0, 1), (1, 3), (3, 5), (5, 8)]
        for k0, k1 in chunks:  # fine pacing for wqk + xt tile 0
            nc.sync.dma_start(
                wqk[:, k0:k1, :], src3(wqkT[128 * k0 : 128 * k1, :], k1 - k0, 512)
            )
            nc.sync.dma_start(
                xt[0][:, k0:k1, :], src3(xT[128 * k0 : 128 * k1, 0:QT], k1 - k0, QT)
            )
        nc.sync.dma_start(wv[:], src3(wvT, N_DIMB, 256))
        for half in range(2):  # xt tile 1 halved: its q chains fill attend(0)
            ks = slice(4 * half, 4 * half + 4)
            nc.sync.dma_start(
                xt[1][:, ks, :],
                src3(xT[512 * half : 512 * half + 512, QT : 2 * QT], 4, QT),
            )
        for ct in range(2, N_QT):
            nc.sync.dma_start(
                xt[ct][:], src3(xT[:, ct * QT : (ct + 1) * QT], N_DIMB, QT)
            )
        nc.sync.dma_start(wo[:], src3(woT, 2, DIM))

        # the ones-columns of every vh tile have no input dependency:
        # emit them all at t=0 while the DVE is otherwise idle
        for tb in range(N_KB):
            nc.vector.memset(vh[tb][:, :, HD : 2 * HD], 1.0)

        def qk_chain(tt, ob):  # ob 0,1 -> q pair blocks; 2,3 -> k pair blocks
            ps = mmps.tile([128, 512], F32, tag="mm512", name=f"qk_ps{tt}_{ob}")
            for kb in range(N_DIMB):
                nc.tensor.matmul(
                    ps[:],
                    wqk[:, kb, ob * 128 : (ob + 1) * 128],
                    xt[tt][:, kb, :],
                    start=(kb == 0),
                    stop=(kb == N_DIMB - 1),
                )
            dest = (qt if ob < 2 else kt)[ob % 2][tt]
            nc.vector.tensor_copy(dest[:], ps[:])

        def v_chain(tb):
            ps = mmps.tile([128, 256], F32, tag="mm512", name=f"v_ps{tb}")
            for kb in range(N_DIMB):
                nc.tensor.matmul(
                    ps[:],
                    xt[tb // 4][:, kb, (tb % 4) * KB : (tb % 4 + 1) * KB],
                    wv[:, kb, :],
                    start=(kb == 0),
                    stop=(kb == N_DIMB - 1),
                )
            nc.vector.tensor_copy(vh[tb][:, :, 0:HD], ps[:])

        def q_pieces(tt):  # q projections: needed before attend(tt) starts
            for ob in range(2):
                yield None, (lambda ob=ob: qk_chain(tt, ob))

        def kv_pieces(tt):
            """k/v projections of tile tt, with deadlines (hp, key-block)
            inside attend(tt) itself: k for head-pair hp is first consumed by
            the S matmul of block 4*tt of that pair; v[b] by the AV matmul of
            block b of pair 0."""
            yield (0, 4 * tt), (lambda: qk_chain(tt, 2))
            for tb in range(4 * tt, 4 * tt + 4):
                yield (0, tb), (lambda tb=tb: v_chain(tb))
            yield (1, 4 * tt), (lambda: qk_chain(tt, 3))

        def attend(tt, os_pair, fillers, deferred, reserve=0, post_hp1=None):
            nb = 4 * (tt + 1)  # allowed key blocks for this query tile
            step = 0
            done_fill = 0
            n_fill = len(fillers) - reserve

            def run_piece():
                nonlocal done_fill
                fillers.popleft()[1]()
                done_fill += 1

            def fill():
                want = min((step + 1) * n_fill // (2 * nb), n_fill)
                while done_fill < want and fillers:
                    run_piece()

            def flush(hp, b):
                # force-run any deadline piece due at or before (hp, b)
                nonlocal done_fill
                if not any(k is not None and k <= (hp, b) for k, _ in fillers):
                    return
                rest = deque()
                while fillers:
                    k, fn = fillers.popleft()
                    if k is not None and k <= (hp, b):
                        fn()
                        done_fill += 1
                    else:
                        rest.append((k, fn))
                fillers.extend(rest)

            for hp in range(2):  # head pair (2hp, 2hp+1)
                ot = [
                    otps.tile([128, QT], F32, tag="ot", name=f"ot{tt}_{hp}_{i}")
                    for i in range(2)
                ]

                # normalize: os_pair[hp][i*64:(i+1)*64] = ot[i][0:64]/ot[i][64]
                # (denominator replicated on partitions 64-127 by the 64
                # ones-columns in vh; DVE ops partition-shift as needed)
                def norm(c0, c1, hp=hp, ot=ot):
                    for i in range(2):
                        rb = rpool.tile(
                            [64, c1 - c0],
                            F32,
                            tag=f"rb{c1 - c0}",
                            name=f"rb{tt}_{hp}_{i}_{c0}",
                        )
                        nc.vector.reciprocal(rb[:], ot[i][64:128, c0:c1])
                        nc.vector.tensor_mul(
                            os_pair[hp][i * 64 : (i + 1) * 64, c0:c1],
                            ot[i][0:64, c0:c1],
                            rb[:],
                        )

                norm_mid = lambda: norm(0, QT // 2)

                def s_mm(b):
                    """S^T for key block b, both heads, into one 2-bank tile."""
                    diag = b - 4 * tt
                    d = diag * 128 if diag >= 0 else 0
                    s = sps.tile([128, 2 * QT], F32, tag="s2", name=f"s{tt}_{hp}_{b}")
                    for i in range(2):
                        rows = slice(i * 64, i * 64 + 64)
                        nc.tensor.matmul(
                            s[:, i * QT + d : (i + 1) * QT],
                            kt[hp][b // 4][rows, (b % 4) * KB : (b % 4 + 1) * KB],
                            qt[hp][tt][rows, d:QT],
                            start=True,
                            stop=True,
                        )
                    p = ppool.tile([128, 2 * QT], F16, tag="p", name=f"p{tt}_{hp}_{b}")
                    return s, p

                flush(hp, 0)
                s_tiles = {0: s_mm(0)}
                for b in range(nb):
                    if b + 1 < nb:
                        flush(hp, b + 1)
                        s_tiles[b + 1] = s_mm(b + 1)
                    diag = b - 4 * tt
                    d = diag * 128 if diag >= 0 else 0
                    s, p = s_tiles.pop(b)
                    if diag < 0:
                        nc.scalar.activation(p[:], s[:], EXP, scale=SCALE)
                    else:
                        # one exp for both heads over cols >= d (all rows),
                        # then zero the masked corner (rows 64-127 of each
                        # head attend only cols >= d+64) AFTER the exp
                        s2 = s[:].rearrange("p (h c) -> p h c", h=2)
                        p2 = p[:].rearrange("p (h c) -> p h c", h=2)
                        nc.scalar.activation(
                            p2[:, :, d:QT], s2[:, :, d:QT], EXP, scale=SCALE
                        )
                        # keep the corner memset off the DVE stream for the
                        # last two blocks of the final pair, where it must
                        # not queue behind the staggered normalize
                        if tt >= N_QT - 2 and b >= nb - 2:
                            nc.gpsimd.memset(p2[64:128, :, d : d + 64], 0.0)
                        else:
                            nc.vector.memset(p2[64:128, :, d : d + 64], 0.0)
                    for i in range(2):
                        nc.tensor.matmul(
                            ot[i][:, d:QT],
                            vh[b][:, 2 * hp + i, :],
                            p[:, i * QT + d : (i + 1) * QT],
                            start=(b == 0),
                            stop=(b == nb - 1),
                        )
                    fill()
                    step += 1
                    if tt >= N_QT - 2 and b == nb - 3:
                        # columns [0:256) of ot are final (the last two,
                        # diagonal, blocks only touch cols >= 256): normalize
                        # them now, overlapping the last attention steps
                        norm_mid()

                # the last normalize is staggered: cols [0:256) were
                # emitted mid-loop, cols [256:512) are deferred into the
                # drain so the first y pieces' DVE copies aren't queued
                # behind the whole chain
                if tt == N_QT - 1 and hp == 1:
                    deferred.append(lambda: norm(QT // 2, QT))
                else:
                    norm(QT // 2, QT) if tt >= N_QT - 2 else norm(0, QT)

                if tt == N_QT - 1 and hp == 1:
                    post_hp1()

            while fillers:
                run_piece()

        def y_pieces_paired(tt, os_pair, tail=False, ys2_cell=None):
            pieces = []
            # during the drain the score PSUM banks are free the moment the
            # last exp has read them: two 2-bank score tiles host four of the
            # eight pieces, so no piece's matmuls ever wait on a prior copy
            ys2 = ys2_cell[0] if tail else None
            for t4 in range(4):
                trows = slice(t4 * 128, (t4 + 1) * 128)
                ysb = ypool.tile([128, DIM], F16, tag="ysb", name=f"ysb{tt}_{t4}")
                for jb in range(2):

                    def piece(t4=t4, jb=jb, ysb=ysb, trows=trows):
                        k = t4 * 2 + jb
                        s2slot = {1: (0, 0), 2: (0, 1), 4: (1, 0), 5: (1, 1)}
                        if tail and k in s2slot:
                            # head-pair-0 half was pre-accumulated in post_hp1
                            j, h = s2slot[k]
                            yps = ys2[j][:, h * 512 : (h + 1) * 512]
                            nc.tensor.matmul(
                                yps[:],
                                os_pair[1][:, trows],
                                wo[:, 1, jb * 512 : (jb + 1) * 512],
                                start=False,
                                stop=True,
                            )
                        else:
                            yps = mmps.tile(
                                [128, 512], F32, tag="mm512", name=f"y_ps{tt}_{t4}_{jb}"
                            )
                            for db in range(2):
                                nc.tensor.matmul(
                                    yps[:],
                                    os_pair[db][:, trows],
                                    wo[:, db, jb * 512 : (jb + 1) * 512],
                                    start=(db == 0),
                                    stop=(db == 1),
                                )
                        dest = ysb[:, jb * 512 : (jb + 1) * 512]
                        rows = slice(tt * QT + t4 * 128, tt * QT + (t4 + 1) * 128)
                        if tail:
                            # drain phase: alternate the PSUM->SBUF copies
                            # between the (idle) ScalarE and the DVE so the
                            # final copy chain halves
                            if (t4 + jb) % 2 == 0:
                                nc.scalar.copy(dest, yps[:])
                            else:
                                nc.vector.tensor_copy(dest, yps[:])
                        else:
                            nc.vector.tensor_copy(dest, yps[:])
                        if jb == 1:
                            nc.sync.dma_start(y[rows, :], ysb[:])

                    pieces.append((None, piece))
            return pieces

        # ---- the pipeline ----
        # Tile 0's q/k chains run kb-interleaved across four concurrent PSUM
        # accumulators (the attention pools are untouched this early), so the
        # in-order PE stream is never blocked behind one chain's wait for the
        # next input chunk.
        pre_ps = [
            mmps.tile([128, 512], F32, tag="mm512", name=f"pre_ps{j}")
            for j in range(2)
        ] + [
            otps.tile([128, QT], F32, tag="ot", name=f"pre_ps{j + 2}")
            for j in range(2)
        ]
        pre_obs = [0, 2, 1, 3]
        for kb in range(N_DIMB):
            for j, ob in enumerate(pre_obs):
                nc.tensor.matmul(
                    pre_ps[j][:],
                    wqk[:, kb, ob * 128 : (ob + 1) * 128],
                    xt[0][:, kb, :],
                    start=(kb == 0),
                    stop=(kb == N_DIMB - 1),
                )
        for j, ob in enumerate(pre_obs):
            dest = (qt if ob < 2 else kt)[ob % 2][0]
            nc.vector.tensor_copy(dest[:], pre_ps[j][:])
        v_chain(0)
        prev_b = None
        for tt in range(N_QT):
            os_pair = [
                ospool.tile([128, QT], F16, tag=f"os{i}", name=f"os{i}_{tt}")
                for i in range(2)
            ]
            nxt = deque()
            if tt + 1 < N_QT:
                nxt.extend(q_pieces(tt + 1))
            if tt:
                late = deque(kv_pieces(tt))
            else:
                late = deque(
                    [((0, tb), (lambda tb=tb: v_chain(tb))) for tb in (1, 2, 3)]
                )
            b = deque(prev_b) if prev_b is not None else deque()
            fillers = deque()
            while late or b or nxt:
                if late:
                    fillers.append(late.popleft())
                if b:
                    fillers.append(b.popleft())
                if late:
                    fillers.append(late.popleft())
                if nxt:
                    fillers.append(nxt.popleft())
            deferred = []
            tail = tt == N_QT - 1
            ys2_cell = [None]

            def post_hp1(os_pair=os_pair, ys2_cell=ys2_cell):
                # the score PSUM ring is done with allocations: claim both
                # tiles for the tail y pieces and pre-run their first
                # (head-pair-0) accumulation half inside the normalize
                # window, when the PE would otherwise idle
                ys2_cell[0] = [
                    sps.tile([128, 2 * QT], F32, tag="s2", name=f"ys2_{j}")
                    for j in range(2)
                ]
                s2slot = {1: (0, 0), 2: (0, 1), 4: (1, 0), 5: (1, 1)}
                for k, (j, h) in s2slot.items():
                    t4, jb = divmod(k, 2)
                    nc.tensor.matmul(
                        ys2_cell[0][j][:, h * 512 : (h + 1) * 512],
                        os_pair[0][:, t4 * 128 : (t4 + 1) * 128],
                        wo[:, 0, jb * 512 : (jb + 1) * 512],
                        start=True,
                        stop=False,
                    )


            attend(tt, os_pair, fillers, deferred, post_hp1=post_hp1 if tail else None)
            prev_b = y_pieces_paired(tt, os_pair, tail=tail, ys2_cell=ys2_cell)
        for idx, (_, piece) in enumerate(prev_b):
            if idx == 4:
                for d in deferred:
                    d()
            piece()


def build():
    global _CACHED_NC
    if _CACHED_NC is not None:
        return _CACHED_NC
    nc = bacc.Bacc(
        "TRN2", target_bir_lowering=False, debug=False, enable_asserts=False
    )
    xT = nc.dram_tensor("xT", [DIM, T], F16, kind="ExternalInput").ap()
    wqkT = nc.dram_tensor("wqkT", [DIM, 512], F16, kind="ExternalInput").ap()
    wvT = nc.dram_tensor("wvT", [DIM, 256], F16, kind="ExternalInput").ap()
    woT = nc.dram_tensor("woutT", [256, DIM], F16, kind="ExternalInput").ap()
    y = nc.dram_tensor("y", [T, DIM], F16, kind="ExternalOutput").ap()
    with tile.TileContext(nc) as tc:
        _emit(nc, tc, xT, wqkT, wvT, woT, y)
    nc.compile()
    _CACHED_NC = nc
    return nc


def make_in_maps(x, Wqkv, Wout):
    """Host-side sharding: core c = (batch c//4, head-group c%4)."""
    in_maps = []
    for c in range(8):
        b, hg = divmod(c, 4)
        hs = hg * H_PER_CORE
        r0, r1 = hs * HD, (hs + H_PER_CORE) * HD
        qrows = Wqkv[r0:r1]
        krows = Wqkv[DIM + r0 : DIM + r1]
        vrows = Wqkv[2 * DIM + r0 : 2 * DIM + r1]
        in_maps.append(
            {
                "xT": np.ascontiguousarray(x[b].T.astype(np.float16)),
                "wqkT": np.ascontiguousarray(
                    np.concatenate([qrows, krows], 0).T.astype(np.float16)
                ),
                "wvT": np.ascontiguousarray(vrows.T.astype(np.float16)),
                "woutT": np.ascontiguousarray(Wout[:, r0:r1].T.astype(np.float16)),
            }
        )
    return in_maps


def kernel(x, Wqkv, Wout):
    x = np.asarray(x, dtype=np.float32)
    Wqkv = np.asarray(Wqkv, dtype=np.float32)
    Wout = np.asarray(Wout, dtype=np.float32)
    nc = build()
    in_maps = make_in_maps(x, Wqkv, Wout)
    res = run_bass_kernel_spmd(nc, in_maps, core_ids=list(range(8)))
    out = np.zeros((B, T, DIM), np.float32)
    for c in range(8):
        out[c // 4] += res.results[c]["y"].astype(np.float32)
    return out


# revision 73
# speedup vs baseline: 1.1754x; 1.0019x over previous
"""Trainium2 Bass kernel for block-causal (chunked) multi-head attention.

Computes, for x:[2,2048,1024], Wqkv:[3072,1024], Wout:[1024,1024]:
    qkv = x @ Wqkv.T ; per-head scaled scores; block-causal mask
    (causal OR same 64-chunk == full attention to all chunks <= own chunk);
    softmax; out = attn @ v ; y = out @ Wout.T

Sharding over 8 NeuronCores: data-parallel over batch (2) x tensor-parallel
over heads (16 heads -> 4 per core).  Each core projects q/k/v for its 4
heads, runs attention, and computes a partial output projection against its
256 columns of Wout; the host sums the 4 partials per batch element.

All SBUF operands are float16 (PE runs f16 at 1 row/cycle with no small-tile
penalty, DMA bytes halve, and the ~1e-3 quantization error is far inside the
tolerance); PSUM accumulation stays f32.

On-chip layout avoids all transposes: the host hands each core
  xT     [1024, 2048]  (x[b] transposed, f16)
  wqkT   [1024, 512]   (Wqkv rows for its 4 heads' q,k -> transposed)
  wvT    [1024, 256]   (v rows transposed)
  woutT  [256, 1024]   (Wout columns for its head-slice, transposed)
Scores are computed transposed (S^T[tk, tq]) so that the attention matmul
P^T -> (attn @ V) needs no transposes, and the softmax denominator comes
for free from a ones-column appended to V.  The block-causal mask is
realized structurally: masked-out key blocks are simply never computed, and
the diagonal blocks use rectangular sub-views (chunk granularity 64).

Engines execute their instruction streams in order, so the emission is a
software pipeline over the 4 query tiles: the TensorE stream for the
(ScalarE-paced) attention of tile t is interleaved with "filler" matmul
chains -- the output projection of tile t-1, the q projections of tile t+1,
and (deadline-scheduled) tile t's OWN k/v projections, which are only
consumed from key-block 4t onward -- keeping the PE busy through every exp
dependency stall.

Head and tail are flattened further: the first projection runs its four
chains kb-interleaved across four PSUM accumulators so the PE is never
queued behind one chain's wait for an input-DMA chunk; the softmax
normalizes of the last tiles are staggered into the attention loop (column
ranges finalize early because later diagonal blocks touch only later
columns); and during the drain the score-PSUM banks are recycled for the
output-projection pieces, whose first accumulation half runs inside the
final normalize window.
"""

import sys

if "/opt/trn_rl_repo" not in sys.path:
    sys.path.insert(0, "/opt/trn_rl_repo")

from collections import deque

import numpy as np

import concourse.bass as bass  # noqa: F401  (registers types)
import concourse.mybir as mybir
import concourse.tile as tile
from concourse import bacc
from concourse.bass_utils import run_bass_kernel_spmd

F32 = mybir.dt.float32
F16 = mybir.dt.float16
EXP = mybir.ActivationFunctionType.Exp

B = 2
T = 2048
DIM = 1024
N_HEADS = 16
HD = 64
CHUNK = 64
H_PER_CORE = 4  # 16 heads / (8 cores / 2 batches)
QT = 512  # query tile (free dim of S^T matmuls)
KB = 128  # key block (contraction block of AV matmuls)
N_QT = T // QT  # 4
N_KB = T // KB  # 16
N_DIMB = DIM // 128  # 8 contraction blocks for the projections
SCALE = 1.0 / np.sqrt(HD)

_CACHED_NC = None


def _emit(nc, tc, xT, wqkT, wvT, woT, y):
    po = tc.tile_pool  # shorthand

    with (
        po(name="persist", bufs=1) as pp,
        po(name="s_ps", bufs=2, space="PSUM") as sps,  # [128,1024] score slots
        po(name="mm_ps", bufs=2, space="PSUM") as mmps,  # [128,512] proj/y slots
        po(name="ot_ps", bufs=2, space="PSUM") as otps,  # [128,512] outT slots
        po(name="pbuf", bufs=6) as ppool,  # exp(S^T) tiles
        po(name="osbuf", bufs=2) as ospool,  # assembled normalized outT
        po(name="rbuf", bufs=2) as rpool,  # reciprocal denominators
        po(name="ybuf", bufs=6) as ypool,
    ):
        # ---- persistent SBUF tensors (kb stacked in the free dim so input
        # DMAs batch into a few large transfers) ----
        xt = [pp.tile([128, N_DIMB, QT], F16, tag=f"xt{c}", name=f"xt{c}") for c in range(N_QT)]
        wqk = pp.tile([128, N_DIMB, 512], F16, tag="wqk", name="wqk")
        wv = pp.tile([128, N_DIMB, 256], F16, tag="wv", name="wv")
        wo = pp.tile([128, 2, DIM], F16, tag="wo", name="wo")
        # q/k head-dim-major: partition block hp holds heads (2hp, 2hp+1)
        qt = [
            [pp.tile([128, QT], F16, tag=f"qt{i}_{c}", name=f"qt{i}_{c}") for c in range(N_QT)]
            for i in range(2)
        ]
        kt = [
            [pp.tile([128, QT], F16, tag=f"kt{i}_{c}", name=f"kt{i}_{c}") for c in range(N_QT)]
            for i in range(2)
        ]
        # v (token-major) + ones columns, per key block: [128, 4 heads, 2*64]
        vh = [
            pp.tile([128, H_PER_CORE, 2 * HD], F16, tag=f"vh{b}", name=f"vh{b}")
            for b in range(N_KB)
        ]

        # ---- input DMAs: tile-0 inputs arrive in fine chunks so the first
        # projection chains start early; the rest are single batched DMAs
        def src3(t, rows, cols):  # [rows*128, cols] -> [128, rows-chunks, cols]
            return t.rearrange("(k p) n -> p k n", p=128)

        chunks = [(0, 1), (1, 3), (3, 5), (5, 8)]
        for k0, k1 in chunks:  # fine pacing for wqk + xt tile 0
            nc.sync.dma_start(
                wqk[:, k0:k1, :], src3(wqkT[128 * k0 : 128 * k1, :], k1 - k0, 512)
            )
            nc.sync.dma_start(
                xt[0][:, k0:k1, :], src3(xT[128 * k0 : 128 * k1, 0:QT], k1 - k0, QT)
            )
        nc.sync.dma_start(wv[:], src3(wvT, N_DIMB, 256))
        for half in range(2):  # xt tile 1 halved: its q chains fill attend(0)
            ks = slice(4 * half, 4 * half + 4)
            nc.sync.dma_start(
                xt[1][:, ks, :],
                src3(xT[512 * half : 512 * half + 512, QT : 2 * QT], 4, QT),
            )
        for ct in range(2, N_QT):
            nc.sync.dma_start(
                xt[ct][:], src3(xT[:, ct * QT : (ct + 1) * QT], N_DIMB, QT)
            )
        nc.sync.dma_start(wo[:], src3(woT, 2, DIM))

        # the ones-columns of every vh tile have no input dependency:
        # emit them all at t=0 while the DVE is otherwise idle
        for tb in range(N_KB):
            nc.vector.memset(vh[tb][:, :, HD : 2 * HD], 1.0)

        def qk_chain(tt, ob):  # ob 0,1 -> q pair blocks; 2,3 -> k pair blocks
            ps = mmps.tile([128, 512], F32, tag="mm512", name=f"qk_ps{tt}_{ob}")
            for kb in range(N_DIMB):
                nc.tensor.matmul(
                    ps[:],
                    wqk[:, kb, ob * 128 : (ob + 1) * 128],
                    xt[tt][:, kb, :],
                    start=(kb == 0),
                    stop=(kb == N_DIMB - 1),
                )
            dest = (qt if ob < 2 else kt)[ob % 2][tt]
            nc.vector.tensor_copy(dest[:], ps[:])

        def v_chain(tb):
            ps = mmps.tile([128, 256], F32, tag="mm512", name=f"v_ps{tb}")
            for kb in range(N_DIMB):
                nc.tensor.matmul(
                    ps[:],
                    xt[tb // 4][:, kb, (tb % 4) * KB : (tb % 4 + 1) * KB],
                    wv[:, kb, :],
                    start=(kb == 0),
                    stop=(kb == N_DIMB - 1),
                )
            nc.vector.tensor_copy(vh[tb][:, :, 0:HD], ps[:])

        def q_pieces(tt):  # q projections: needed before attend(tt) starts
            for ob in range(2):
                yield None, (lambda ob=ob: qk_chain(tt, ob))

        def kv_pieces(tt):
            """k/v projections of tile tt, with deadlines (hp, key-block)
            inside attend(tt) itself: k for head-pair hp is first consumed by
            the S matmul of block 4*tt of that pair; v[b] by the AV matmul of
            block b of pair 0."""
            yield (0, 4 * tt), (lambda: qk_chain(tt, 2))
            for tb in range(4 * tt, 4 * tt + 4):
                yield (0, tb), (lambda tb=tb: v_chain(tb))
            yield (1, 4 * tt), (lambda: qk_chain(tt, 3))

        def attend(tt, os_pair, fillers, deferred, reserve=0, post_hp1=None):
            nb = 4 * (tt + 1)  # allowed key blocks for this query tile
            step = 0
            done_fill = 0
            n_fill = len(fillers) - reserve

            def run_piece():
                nonlocal done_fill
                fillers.popleft()[1]()
                done_fill += 1

            def fill():
                want = min((step + 1) * n_fill // (2 * nb), n_fill)
                while done_fill < want and fillers:
                    run_piece()

            def flush(hp, b):
                # force-run any deadline piece due at or before (hp, b)
                nonlocal done_fill
                if not any(k is not None and k <= (hp, b) for k, _ in fillers):
                    return
                rest = deque()
                while fillers:
                    k, fn = fillers.popleft()
                    if k is not None and k <= (hp, b):
                        fn()
                        done_fill += 1
                    else:
                        rest.append((k, fn))
                fillers.extend(rest)

            for hp in range(2):  # head pair (2hp, 2hp+1)
                ot = [
                    otps.tile([128, QT], F32, tag="ot", name=f"ot{tt}_{hp}_{i}")
                    for i in range(2)
                ]

                # normalize: os_pair[hp][i*64:(i+1)*64] = ot[i][0:64]/ot[i][64]
                # (denominator replicated on partitions 64-127 by the 64
                # ones-columns in vh; DVE ops partition-shift as needed)
                def norm(c0, c1, hp=hp, ot=ot):
                    for i in range(2):
                        rb = rpool.tile(
                            [64, c1 - c0],
                            F32,
                            tag=f"rb{c1 - c0}",
                            name=f"rb{tt}_{hp}_{i}_{c0}",
                        )
                        nc.vector.reciprocal(rb[:], ot[i][64:128, c0:c1])
                        nc.vector.tensor_mul(
                            os_pair[hp][i * 64 : (i + 1) * 64, c0:c1],
                            ot[i][0:64, c0:c1],
                            rb[:],
                        )

                norm_mid = lambda: norm(0, QT // 2)

                def s_mm(b):
                    """S^T for key block b, both heads, into one 2-bank tile."""
                    diag = b - 4 * tt
                    d = diag * 128 if diag >= 0 else 0
                    s = sps.tile([128, 2 * QT], F32, tag="s2", name=f"s{tt}_{hp}_{b}")
                    for i in range(2):
                        rows = slice(i * 64, i * 64 + 64)
                        nc.tensor.matmul(
                            s[:, i * QT + d : (i + 1) * QT],
                            kt[hp][b // 4][rows, (b % 4) * KB : (b % 4 + 1) * KB],
                            qt[hp][tt][rows, d:QT],
                            start=True,
                            stop=True,
                        )
                    p = ppool.tile([128, 2 * QT], F16, tag="p", name=f"p{tt}_{hp}_{b}")
                    return s, p

                flush(hp, 0)
                s_tiles = {0: s_mm(0)}
                for b in range(nb):
                    if b + 1 < nb:
                        flush(hp, b + 1)
                        s_tiles[b + 1] = s_mm(b + 1)
                    diag = b - 4 * tt
                    d = diag * 128 if diag >= 0 else 0
                    s, p = s_tiles.pop(b)
                    if diag < 0:
                        nc.scalar.activation(p[:], s[:], EXP, scale=SCALE)
                    else:
                        # one exp for both heads over cols >= d (all rows),
                        # then zero the masked corner (rows 64-127 of each
                        # head attend only cols >= d+64) AFTER the exp
                        s2 = s[:].rearrange("p (h c) -> p h c", h=2)
                        p2 = p[:].rearrange("p (h c) -> p h c", h=2)
                        nc.scalar.activation(
                            p2[:, :, d:QT], s2[:, :, d:QT], EXP, scale=SCALE
                        )
                        # keep the corner memset off the DVE stream for the
                        # last two blocks of the final pair, where it must
                        # not queue behind the staggered normalize
                        if tt >= N_QT - 2 and b >= nb - 2:
                            nc.gpsimd.memset(p2[64:128, :, d : d + 64], 0.0)
                        else:
                            nc.vector.memset(p2[64:128, :, d : d + 64], 0.0)
                    for i in range(2):
                        nc.tensor.matmul(
                            ot[i][:, d:QT],
                            vh[b][:, 2 * hp + i, :],
                            p[:, i * QT + d : (i + 1) * QT],
                            start=(b == 0),
                            stop=(b == nb - 1),
                        )
                    fill()
                    step += 1
                    if tt >= N_QT - 2 and b == nb - 3:
                        # columns [0:256) of ot are final (the last two,
                        # diagonal, blocks only touch cols >= 256): normalize
                        # them now, overlapping the last attention steps
                        norm_mid()

                # the last normalize is staggered: cols [0:256) were
                # emitted mid-loop, cols [256:512) are deferred into the
                # drain so the first y pieces' DVE copies aren't queued
                # behind the whole chain
                if tt == N_QT - 1 and hp == 1:
                    deferred.append(lambda: norm(QT // 2, QT))
                else:
                    norm(QT // 2, QT) if tt >= N_QT - 2 else norm(0, QT)

                if tt == N_QT - 1 and hp == 1:
                    post_hp1()

            while fillers:
                run_piece()

        def y_pieces_paired(tt, os_pair, tail=False, ys2_cell=None):
            pieces = []
            # during the drain the score PSUM banks are free the moment the
            # last exp has read them: two 2-bank score tiles host four of the
            # eight pieces, so no piece's matmuls ever wait on a prior copy
            ys2 = ys2_cell[0] if tail else None
            for t4 in range(4):
                trows = slice(t4 * 128, (t4 + 1) * 128)
                ysb = ypool.tile([128, DIM], F16, tag="ysb", name=f"ysb{tt}_{t4}")
                for jb in range(2):

                    def piece(t4=t4, jb=jb, ysb=ysb, trows=trows):
                        k = t4 * 2 + jb
                        s2slot = {1: (0, 0), 2: (0, 1), 4: (1, 0), 5: (1, 1)}
                        if tail and k in s2slot:
                            # head-pair-0 half was pre-accumulated in post_hp1
                            j, h = s2slot[k]
                            yps = ys2[j][:, h * 512 : (h + 1) * 512]
                            nc.tensor.matmul(
                                yps[:],
                                os_pair[1][:, trows],
                                wo[:, 1, jb * 512 : (jb + 1) * 512],
                                start=False,
                                stop=True,
                            )
                        else:
                            yps = mmps.tile(
                                [128, 512], F32, tag="mm512", name=f"y_ps{tt}_{t4}_{jb}"
                            )
                            for db in range(2):
                                nc.tensor.matmul(
                                    yps[:],
                                    os_pair[db][:, trows],
                                    wo[:, db, jb * 512 : (jb + 1) * 512],
                                    start=(db == 0),
                                    stop=(db == 1),
                                )
                        dest = ysb[:, jb * 512 : (jb + 1) * 512]
                        rows = slice(tt * QT + t4 * 128, tt * QT + (t4 + 1) * 128)
                        if tail:
                            # drain phase: alternate the PSUM->SBUF copies
                            # between the (idle) ScalarE and the DVE so the
                            # final copy chain halves
                            if (t4 + jb) % 2 == 0:
                                nc.scalar.copy(dest, yps[:])
                            else:
                                nc.vector.tensor_copy(dest, yps[:])
                        else:
                            nc.vector.tensor_copy(dest, yps[:])
                        if jb == 1:
                            nc.sync.dma_start(y[rows, :], ysb[:])

                    pieces.append((None, piece))
            return pieces

        # ---- the pipeline ----
        # Tile 0's q/k chains run kb-interleaved across four concurrent PSUM
        # accumulators (the attention pools are untouched this early), so the
        # in-order PE stream is never blocked behind one chain's wait for the
        # next input chunk.
        pre_ps = [
            mmps.tile([128, 512], F32, tag="mm512", name=f"pre_ps{j}")
            for j in range(2)
        ] + [
            otps.tile([128, QT], F32, tag="ot", name=f"pre_ps{j + 2}")
            for j in range(2)
        ]
        pre_obs = [0, 2, 1, 3]
        for kb in range(N_DIMB):
            for j, ob in enumerate(pre_obs):
                nc.tensor.matmul(
                    pre_ps[j][:],
                    wqk[:, kb, ob * 128 : (ob + 1) * 128],
                    xt[0][:, kb, :],
                    start=(kb == 0),
                    stop=(kb == N_DIMB - 1),
                )
        for j, ob in enumerate(pre_obs):
            dest = (qt if ob < 2 else kt)[ob % 2][0]
            nc.vector.tensor_copy(dest[:], pre_ps[j][:])
        v_chain(0)
        prev_b = None
        for tt in range(N_QT):
            os_pair = [
                ospool.tile([128, QT], F16, tag=f"os{i}", name=f"os{i}_{tt}")
                for i in range(2)
            ]
            nxt = deque()
            if tt + 1 < N_QT:
                nxt.extend(q_pieces(tt + 1))
            if tt:
                late = deque(kv_pieces(tt))
            else:
                late = deque(
                    [((0, tb), (lambda tb=tb: v_chain(tb))) for tb in (1, 2, 3)]
                )
            b = deque(prev_b) if prev_b is not None else deque()
            fillers = deque()
            while late or b or nxt:
                if late:
                    fillers.append(late.popleft())
                if b:
                    fillers.append(b.popleft())
                if late:
                    fillers.append(late.popleft())
                if nxt:
                    fillers.append(nxt.popleft())
            deferred = []
            tail = tt == N_QT - 1
            ys2_cell = [None]

            def post_hp1(os_pair=os_pair, ys2_cell=ys2_cell):
                # the score PSUM ring is done with allocations: claim both
                # tiles for the tail y pieces and pre-run their first
                # (head-pair-0) accumulation half inside the normalize
                # window, when the PE would otherwise idle
                ys2_cell[0] = [
                    sps.tile([128, 2 * QT], F32, tag="s2", name=f"ys2_{j}")
                    for j in range(2)
                ]
                s2slot = {1: (0, 0), 2: (0, 1), 4: (1, 0), 5: (1, 1)}
                for k, (j, h) in s2slot.items():
                    t4, jb = divmod(k, 2)
                    nc.tensor.matmul(
                        ys2_cell[0][j][:, h * 512 : (h + 1) * 512],
                        os_pair[0][:, t4 * 128 : (t4 + 1) * 128],
                        wo[:, 0, jb * 512 : (jb + 1) * 512],
                        start=True,
                        stop=False,
                    )


            attend(tt, os_pair, fillers, deferred, post_hp1=post_hp1 if tail else None)
            prev_b = y_pieces_paired(tt, os_pair, tail=tail, ys2_cell=ys2_cell)
        for idx, (_, piece) in enumerate(prev_b):
            if idx == 4:
                for d in deferred:
                    d()
            piece()


def build():
    global _CACHED_NC
    if _CACHED_NC is not None:
        return _CACHED_NC
    nc = bacc.Bacc(
        "TRN2", target_bir_lowering=False, debug=False, enable_asserts=False
    )
    xT = nc.dram_tensor("xT", [DIM, T], F16, kind="ExternalInput").ap()
    wqkT = nc.dram_tensor("wqkT", [DIM, 512], F16, kind="ExternalInput").ap()
    wvT = nc.dram_tensor("wvT", [DIM, 256], F16, kind="ExternalInput").ap()
    woT = nc.dram_tensor("woutT", [256, DIM], F16, kind="ExternalInput").ap()
    y = nc.dram_tensor("y", [T, DIM], F16, kind="ExternalOutput").ap()
    with tile.TileContext(nc) as tc:
        _emit(nc, tc, xT, wqkT, wvT, woT, y)
    nc.compile()
    _CACHED_NC = nc
    return nc


def make_in_maps(x, Wqkv, Wout):
    """Host-side sharding: core c = (batch c//4, head-group c%4)."""
    in_maps = []
    for c in range(8):
        b, hg = divmod(c, 4)
        hs = hg * H_PER_CORE
        r0, r1 = hs * HD, (hs + H_PER_CORE) * HD
        qrows = Wqkv[r0:r1]
        krows = Wqkv[DIM + r0 : DIM + r1]
        vrows = Wqkv[2 * DIM + r0 : 2 * DIM + r1]
        in_maps.append(
            {
                "xT": np.ascontiguousarray(x[b].T.astype(np.float16)),
                "wqkT": np.ascontiguousarray(
                    np.concatenate([qrows, krows], 0).T.astype(np.float16)
                ),
                "wvT": np.ascontiguousarray(vrows.T.astype(np.float16)),
                "woutT": np.ascontiguousarray(Wout[:, r0:r1].T.astype(np.float16)),
            }
        )
    return in_maps


def kernel(x, Wqkv, Wout):
    x = np.asarray(x, dtype=np.float32)
    Wqkv = np.asarray(Wqkv, dtype=np.float32)
    Wout = np.asarray(Wout, dtype=np.float32)
    nc = build()
    in_maps = make_in_maps(x, Wqkv, Wout)
    res = run_bass_kernel_spmd(nc, in_maps, core_ids=list(range(8)))
    out = np.zeros((B, T, DIM), np.float32)
    for c in range(8):
        out[c // 4] += res.results[c]["y"].astype(np.float32)
    return out


# revision 75
# speedup vs baseline: 1.1764x; 1.0008x over previous
"""Trainium2 Bass kernel for block-causal (chunked) multi-head attention.

Computes, for x:[2,2048,1024], Wqkv:[3072,1024], Wout:[1024,1024]:
    qkv = x @ Wqkv.T ; per-head scaled scores; block-causal mask
    (causal OR same 64-chunk == full attention to all chunks <= own chunk);
    softmax; out = attn @ v ; y = out @ Wout.T

Sharding over 8 NeuronCores: data-parallel over batch (2) x tensor-parallel
over heads (16 heads -> 4 per core).  Each core projects q/k/v for its 4
heads, runs attention, and computes a partial output projection against its
256 columns of Wout; the host sums the 4 partials per batch element.

All SBUF operands are float16 (PE runs f16 at 1 row/cycle with no small-tile
penalty, DMA bytes halve, and the ~1e-3 quantization error is far inside the
tolerance); PSUM accumulation stays f32.

On-chip layout avoids all transposes: the host hands each core
  xT     [1024, 2048]  (x[b] transposed, f16)
  wqkT   [1024, 512]   (Wqkv rows for its 4 heads' q,k -> transposed)
  wvT    [1024, 256]   (v rows transposed)
  woutT  [256, 1024]   (Wout columns for its head-slice, transposed)
Scores are computed transposed (S^T[tk, tq]) so that the attention matmul
P^T -> (attn @ V) needs no transposes, and the softmax denominator comes
for free from a ones-column appended to V.  The block-causal mask is
realized structurally: masked-out key blocks are simply never computed, and
the diagonal blocks use rectangular sub-views (chunk granularity 64).

Engines execute their instruction streams in order, so the emission is a
software pipeline over the 4 query tiles: the TensorE stream for the
(ScalarE-paced) attention of tile t is interleaved with "filler" matmul
chains -- the output projection of tile t-1, the q projections of tile t+1,
and (deadline-scheduled) tile t's OWN k/v projections, which are only
consumed from key-block 4t onward -- keeping the PE busy through every exp
dependency stall.

Head and tail are flattened further: the first projection runs its four
chains kb-interleaved across four PSUM accumulators so the PE is never
queued behind one chain's wait for an input-DMA chunk; the softmax
normalizes of the last tiles are staggered into the attention loop (column
ranges finalize early because later diagonal blocks touch only later
columns); and during the drain the score-PSUM banks are recycled for the
output-projection pieces, whose first accumulation half runs inside the
final normalize window.
"""

import sys

if "/opt/trn_rl_repo" not in sys.path:
    sys.path.insert(0, "/opt/trn_rl_repo")

from collections import deque

import numpy as np

import concourse.bass as bass  # noqa: F401  (registers types)
import concourse.mybir as mybir
import concourse.tile as tile
from concourse import bacc
from concourse.bass_utils import run_bass_kernel_spmd

F32 = mybir.dt.float32
F16 = mybir.dt.float16
EXP = mybir.ActivationFunctionType.Exp

B = 2
T = 2048
DIM = 1024
N_HEADS = 16
HD = 64
CHUNK = 64
H_PER_CORE = 4  # 16 heads / (8 cores / 2 batches)
QT = 512  # query tile (free dim of S^T matmuls)
KB = 128  # key block (contraction block of AV matmuls)
N_QT = T // QT  # 4
N_KB = T // KB  # 16
N_DIMB = DIM // 128  # 8 contraction blocks for the projections
SCALE = 1.0 / np.sqrt(HD)

_CACHED_NC = None


def _emit(nc, tc, xT, wqkT, wvT, woT, y):
    po = tc.tile_pool  # shorthand

    with (
        po(name="persist", bufs=1) as pp,
        po(name="s_ps", bufs=2, space="PSUM") as sps,  # [128,1024] score slots
        po(name="mm_ps", bufs=2, space="PSUM") as mmps,  # [128,512] proj/y slots
        po(name="ot_ps", bufs=2, space="PSUM") as otps,  # [128,512] outT slots
        po(name="pbuf", bufs=6) as ppool,  # exp(S^T) tiles
        po(name="osbuf", bufs=2) as ospool,  # assembled normalized outT
        po(name="rbuf", bufs=2) as rpool,  # reciprocal denominators
        po(name="ybuf", bufs=6) as ypool,
    ):
        # ---- persistent SBUF tensors (kb stacked in the free dim so input
        # DMAs batch into a few large transfers) ----
        xt = [pp.tile([128, N_DIMB, QT], F16, tag=f"xt{c}", name=f"xt{c}") for c in range(N_QT)]
        wqk = pp.tile([128, N_DIMB, 512], F16, tag="wqk", name="wqk")
        wv = pp.tile([128, N_DIMB, 256], F16, tag="wv", name="wv")
        wo = pp.tile([128, 2, DIM], F16, tag="wo", name="wo")
        # q/k head-dim-major: partition block hp holds heads (2hp, 2hp+1)
        qt = [
            [pp.tile([128, QT], F16, tag=f"qt{i}_{c}", name=f"qt{i}_{c}") for c in range(N_QT)]
            for i in range(2)
        ]
        kt = [
            [pp.tile([128, QT], F16, tag=f"kt{i}_{c}", name=f"kt{i}_{c}") for c in range(N_QT)]
            for i in range(2)
        ]
        # v (token-major) + ones columns, per key block: [128, 4 heads, 2*64]
        vh = [
            pp.tile([128, H_PER_CORE, 2 * HD], F16, tag=f"vh{b}", name=f"vh{b}")
            for b in range(N_KB)
        ]

        # ---- input DMAs: tile-0 inputs arrive in fine chunks so the first
        # projection chains start early; the rest are single batched DMAs
        def src3(t, rows, cols):  # [rows*128, cols] -> [128, rows-chunks, cols]
            return t.rearrange("(k p) n -> p k n", p=128)

        chunks = [(0, 1), (1, 3), (3, 5), (5, 8)]
        for k0, k1 in chunks:  # fine pacing for wqk + xt tile 0
            nc.sync.dma_start(
                wqk[:, k0:k1, :], src3(wqkT[128 * k0 : 128 * k1, :], k1 - k0, 512)
            )
            nc.sync.dma_start(
                xt[0][:, k0:k1, :], src3(xT[128 * k0 : 128 * k1, 0:QT], k1 - k0, QT)
            )
        nc.sync.dma_start(wv[:], src3(wvT, N_DIMB, 256))
        for half in range(2):  # xt tile 1 halved: its q chains fill attend(0)
            ks = slice(4 * half, 4 * half + 4)
            nc.sync.dma_start(
                xt[1][:, ks, :],
                src3(xT[512 * half : 512 * half + 512, QT : 2 * QT], 4, QT),
            )
        for half in range(2):
            ks = slice(4 * half, 4 * half + 4)
            nc.sync.dma_start(
                xt[2][:, ks, :],
                src3(xT[512 * half : 512 * half + 512, 2 * QT : 3 * QT], 4, QT),
            )
        nc.sync.dma_start(wo[:], src3(woT, 2, DIM))
        for half in range(2):
            ks = slice(4 * half, 4 * half + 4)
            nc.sync.dma_start(
                xt[3][:, ks, :],
                src3(xT[512 * half : 512 * half + 512, 3 * QT : 4 * QT], 4, QT),
            )

        # the ones-columns of every vh tile have no input dependency:
        # emit them all at t=0 while the DVE is otherwise idle
        for tb in range(N_KB):
            nc.vector.memset(vh[tb][:, :, HD : 2 * HD], 1.0)

        def qk_chain(tt, ob):  # ob 0,1 -> q pair blocks; 2,3 -> k pair blocks
            ps = mmps.tile([128, 512], F32, tag="mm512", name=f"qk_ps{tt}_{ob}")
            for kb in range(N_DIMB):
                nc.tensor.matmul(
                    ps[:],
                    wqk[:, kb, ob * 128 : (ob + 1) * 128],
                    xt[tt][:, kb, :],
                    start=(kb == 0),
                    stop=(kb == N_DIMB - 1),
                )
            dest = (qt if ob < 2 else kt)[ob % 2][tt]
            nc.vector.tensor_copy(dest[:], ps[:])

        def v_chain(tb):
            ps = mmps.tile([128, 256], F32, tag="mm512", name=f"v_ps{tb}")
            for kb in range(N_DIMB):
                nc.tensor.matmul(
                    ps[:],
                    xt[tb // 4][:, kb, (tb % 4) * KB : (tb % 4 + 1) * KB],
                    wv[:, kb, :],
                    start=(kb == 0),
                    stop=(kb == N_DIMB - 1),
                )
            nc.vector.tensor_copy(vh[tb][:, :, 0:HD], ps[:])

        def q_pieces(tt):  # q projections: needed before attend(tt) starts
            for ob in range(2):
                yield None, (lambda ob=ob: qk_chain(tt, ob))

        def kv_pieces(tt):
            """k/v projections of tile tt, with deadlines (hp, key-block)
            inside attend(tt) itself: k for head-pair hp is first consumed by
            the S matmul of block 4*tt of that pair; v[b] by the AV matmul of
            block b of pair 0."""
            yield (0, 4 * tt), (lambda: qk_chain(tt, 2))
            for tb in range(4 * tt, 4 * tt + 4):
                yield (0, tb), (lambda tb=tb: v_chain(tb))
            yield (1, 4 * tt), (lambda: qk_chain(tt, 3))

        def attend(tt, os_pair, fillers, deferred, reserve=0, post_hp1=None):
            nb = 4 * (tt + 1)  # allowed key blocks for this query tile
            step = 0
            done_fill = 0
            n_fill = len(fillers) - reserve

            def run_piece():
                nonlocal done_fill
                fillers.popleft()[1]()
                done_fill += 1

            def fill():
                want = min((step + 1) * n_fill // (2 * nb), n_fill)
                while done_fill < want and fillers:
                    run_piece()

            def flush(hp, b):
                # force-run any deadline piece due at or before (hp, b)
                nonlocal done_fill
                if not any(k is not None and k <= (hp, b) for k, _ in fillers):
                    return
                rest = deque()
                while fillers:
                    k, fn = fillers.popleft()
                    if k is not None and k <= (hp, b):
                        fn()
                        done_fill += 1
                    else:
                        rest.append((k, fn))
                fillers.extend(rest)

            for hp in range(2):  # head pair (2hp, 2hp+1)
                ot = [
                    otps.tile([128, QT], F32, tag="ot", name=f"ot{tt}_{hp}_{i}")
                    for i in range(2)
                ]

                # normalize: os_pair[hp][i*64:(i+1)*64] = ot[i][0:64]/ot[i][64]
                # (denominator replicated on partitions 64-127 by the 64
                # ones-columns in vh; DVE ops partition-shift as needed)
                def norm(c0, c1, hp=hp, ot=ot):
                    for i in range(2):
                        rb = rpool.tile(
                            [64, c1 - c0],
                            F32,
                            tag=f"rb{c1 - c0}",
                            name=f"rb{tt}_{hp}_{i}_{c0}",
                        )
                        nc.vector.reciprocal(rb[:], ot[i][64:128, c0:c1])
                        nc.vector.tensor_mul(
                            os_pair[hp][i * 64 : (i + 1) * 64, c0:c1],
                            ot[i][0:64, c0:c1],
                            rb[:],
                        )

                norm_mid = lambda: norm(0, QT // 2)

                def s_mm(b):
                    """S^T for key block b, both heads, into one 2-bank tile."""
                    diag = b - 4 * tt
                    d = diag * 128 if diag >= 0 else 0
                    s = sps.tile([128, 2 * QT], F32, tag="s2", name=f"s{tt}_{hp}_{b}")
                    for i in range(2):
                        rows = slice(i * 64, i * 64 + 64)
                        nc.tensor.matmul(
                            s[:, i * QT + d : (i + 1) * QT],
                            kt[hp][b // 4][rows, (b % 4) * KB : (b % 4 + 1) * KB],
                            qt[hp][tt][rows, d:QT],
                            start=True,
                            stop=True,
                        )
                    p = ppool.tile([128, 2 * QT], F16, tag="p", name=f"p{tt}_{hp}_{b}")
                    return s, p

                flush(hp, 0)
                s_tiles = {0: s_mm(0)}
                for b in range(nb):
                    if b + 1 < nb:
                        flush(hp, b + 1)
                        s_tiles[b + 1] = s_mm(b + 1)
                    diag = b - 4 * tt
                    d = diag * 128 if diag >= 0 else 0
                    s, p = s_tiles.pop(b)
                    if diag < 0:
                        nc.scalar.activation(p[:], s[:], EXP, scale=SCALE)
                    else:
                        # one exp for both heads over cols >= d (all rows),
                        # then zero the masked corner (rows 64-127 of each
                        # head attend only cols >= d+64) AFTER the exp
                        s2 = s[:].rearrange("p (h c) -> p h c", h=2)
                        p2 = p[:].rearrange("p (h c) -> p h c", h=2)
                        nc.scalar.activation(
                            p2[:, :, d:QT], s2[:, :, d:QT], EXP, scale=SCALE
                        )
                        # keep the corner memset off the DVE stream for the
                        # last two blocks of the final pair, where it must
                        # not queue behind the staggered normalize
                        if tt >= N_QT - 2 and b >= nb - 2:
                            nc.gpsimd.memset(p2[64:128, :, d : d + 64], 0.0)
                        else:
                            nc.vector.memset(p2[64:128, :, d : d + 64], 0.0)
                    for i in range(2):
                        nc.tensor.matmul(
                            ot[i][:, d:QT],
                            vh[b][:, 2 * hp + i, :],
                            p[:, i * QT + d : (i + 1) * QT],
                            start=(b == 0),
                            stop=(b == nb - 1),
                        )
                    fill()
                    step += 1
                    if tt >= N_QT - 2 and b == nb - 3:
                        # columns [0:256) of ot are final (the last two,
                        # diagonal, blocks only touch cols >= 256): normalize
                        # them now, overlapping the last attention steps
                        norm_mid()

                # the last normalize is staggered: cols [0:256) were
                # emitted mid-loop, cols [256:512) are deferred into the
                # drain so the first y pieces' DVE copies aren't queued
                # behind the whole chain
                if tt == N_QT - 1 and hp == 1:
                    deferred.append(lambda: norm(QT // 2, QT))
                else:
                    norm(QT // 2, QT) if tt >= N_QT - 2 else norm(0, QT)

                if tt == N_QT - 1 and hp == 1:
                    post_hp1()

            while fillers:
                run_piece()

        def y_pieces_paired(tt, os_pair, tail=False, ys2_cell=None):
            pieces = []
            # during the drain the score PSUM banks are free the moment the
            # last exp has read them: two 2-bank score tiles host four of the
            # eight pieces, so no piece's matmuls ever wait on a prior copy
            ys2 = ys2_cell[0] if tail else None
            for t4 in range(4):
                trows = slice(t4 * 128, (t4 + 1) * 128)
                ysb = ypool.tile([128, DIM], F16, tag="ysb", name=f"ysb{tt}_{t4}")
                for jb in range(2):

                    def piece(t4=t4, jb=jb, ysb=ysb, trows=trows):
                        k = t4 * 2 + jb
                        s2slot = {1: (0, 0), 2: (0, 1), 4: (1, 0), 5: (1, 1)}
                        if tail and k in s2slot:
                            # head-pair-0 half was pre-accumulated in post_hp1
                            j, h = s2slot[k]
                            yps = ys2[j][:, h * 512 : (h + 1) * 512]
                            nc.tensor.matmul(
                                yps[:],
                                os_pair[1][:, trows],
                                wo[:, 1, jb * 512 : (jb + 1) * 512],
                                start=False,
                                stop=True,
                            )
                        else:
                            yps = mmps.tile(
                                [128, 512], F32, tag="mm512", name=f"y_ps{tt}_{t4}_{jb}"
                            )
                            for db in range(2):
                                nc.tensor.matmul(
                                    yps[:],
                                    os_pair[db][:, trows],
                                    wo[:, db, jb * 512 : (jb + 1) * 512],
                                    start=(db == 0),
                                    stop=(db == 1),
                                )
                        dest = ysb[:, jb * 512 : (jb + 1) * 512]
                        rows = slice(tt * QT + t4 * 128, tt * QT + (t4 + 1) * 128)
                        if tail:
                            # drain phase: alternate the PSUM->SBUF copies
                            # between the (idle) ScalarE and the DVE so the
                            # final copy chain halves
                            if (t4 + jb) % 2 == 0:
                                nc.scalar.copy(dest, yps[:])
                            else:
                                nc.vector.tensor_copy(dest, yps[:])
                        else:
                            nc.vector.tensor_copy(dest, yps[:])
                        if jb == 1:
                            nc.sync.dma_start(y[rows, :], ysb[:])

                    pieces.append((None, piece))
            return pieces

        # ---- the pipeline ----
        # Tile 0's q/k chains run kb-interleaved across four concurrent PSUM
        # accumulators (the attention pools are untouched this early), so the
        # in-order PE stream is never blocked behind one chain's wait for the
        # next input chunk.
        pre_ps = [
            mmps.tile([128, 512], F32, tag="mm512", name=f"pre_ps{j}")
            for j in range(2)
        ] + [
            otps.tile([128, QT], F32, tag="ot", name=f"pre_ps{j + 2}")
            for j in range(2)
        ]
        pre_obs = [0, 2, 1, 3]
        for kb in range(N_DIMB):
            for j, ob in enumerate(pre_obs):
                nc.tensor.matmul(
                    pre_ps[j][:],
                    wqk[:, kb, ob * 128 : (ob + 1) * 128],
                    xt[0][:, kb, :],
                    start=(kb == 0),
                    stop=(kb == N_DIMB - 1),
                )
        for j, ob in enumerate(pre_obs):
            dest = (qt if ob < 2 else kt)[ob % 2][0]
            nc.vector.tensor_copy(dest[:], pre_ps[j][:])
        v_chain(0)
        prev_b = None
        for tt in range(N_QT):
            os_pair = [
                ospool.tile([128, QT], F16, tag=f"os{i}", name=f"os{i}_{tt}")
                for i in range(2)
            ]
            nxt = deque()
            if tt + 1 < N_QT:
                nxt.extend(q_pieces(tt + 1))
            if tt:
                late = deque(kv_pieces(tt))
            else:
                late = deque(
                    [((0, tb), (lambda tb=tb: v_chain(tb))) for tb in (1, 2, 3)]
                )
            b = deque(prev_b) if prev_b is not None else deque()
            fillers = deque()
            while late or b or nxt:
                if late:
                    fillers.append(late.popleft())
                if b:
                    fillers.append(b.popleft())
                if late:
                    fillers.append(late.popleft())
                if nxt:
                    fillers.append(nxt.popleft())
            deferred = []
            tail = tt == N_QT - 1
            ys2_cell = [None]

            def post_hp1(os_pair=os_pair, ys2_cell=ys2_cell):
                # the score PSUM ring is done with allocations: claim both
                # tiles for the tail y pieces and pre-run their first
                # (head-pair-0) accumulation half inside the normalize
                # window, when the PE would otherwise idle
                ys2_cell[0] = [
                    sps.tile([128, 2 * QT], F32, tag="s2", name=f"ys2_{j}")
                    for j in range(2)
                ]
                s2slot = {1: (0, 0), 2: (0, 1), 4: (1, 0), 5: (1, 1)}
                for k, (j, h) in s2slot.items():
                    t4, jb = divmod(k, 2)
                    nc.tensor.matmul(
                        ys2_cell[0][j][:, h * 512 : (h + 1) * 512],
                        os_pair[0][:, t4 * 128 : (t4 + 1) * 128],
                        wo[:, 0, jb * 512 : (jb + 1) * 512],
                        start=True,
                        stop=False,
                    )


            attend(tt, os_pair, fillers, deferred, post_hp1=post_hp1 if tail else None)
            prev_b = y_pieces_paired(tt, os_pair, tail=tail, ys2_cell=ys2_cell)
        for idx, (_, piece) in enumerate(prev_b):
            if idx == 4:
                for d in deferred:
                    d()
            piece()


def build():
    global _CACHED_NC
    if _CACHED_NC is not None:
        return _CACHED_NC
    nc = bacc.Bacc(
        "TRN2", target_bir_lowering=False, debug=False, enable_asserts=False
    )
    xT = nc.dram_tensor("xT", [DIM, T], F16, kind="ExternalInput").ap()
    wqkT = nc.dram_tensor("wqkT", [DIM, 512], F16, kind="ExternalInput").ap()
    wvT = nc.dram_tensor("wvT", [DIM, 256], F16, kind="ExternalInput").ap()
    woT = nc.dram_tensor("woutT", [256, DIM], F16, kind="ExternalInput").ap()
    y = nc.dram_tensor("y", [T, DIM], F16, kind="ExternalOutput").ap()
    with tile.TileContext(nc) as tc:
        _emit(nc, tc, xT, wqkT, wvT, woT, y)
    nc.compile()
    _CACHED_NC = nc
    return nc


def make_in_maps(x, Wqkv, Wout):
    """Host-side sharding: core c = (batch c//4, head-group c%4)."""
    in_maps = []
    for c in range(8):
        b, hg = divmod(c, 4)
        hs = hg * H_PER_CORE
        r0, r1 = hs * HD, (hs + H_PER_CORE) * HD
        qrows = Wqkv[r0:r1]
        krows = Wqkv[DIM + r0 : DIM + r1]
        vrows = Wqkv[2 * DIM + r0 : 2 * DIM + r1]
        in_maps.append(
            {
                "xT": np.ascontiguousarray(x[b].T.astype(np.float16)),
                "wqkT": np.ascontiguousarray(
                    np.concatenate([qrows, krows], 0).T.astype(np.float16)
                ),
                "wvT": np.ascontiguousarray(vrows.T.astype(np.float16)),
                "woutT": np.ascontiguousarray(Wout[:, r0:r1].T.astype(np.float16)),
            }
        )
    return in_maps


def kernel(x, Wqkv, Wout):
    x = np.asarray(x, dtype=np.float32)
    Wqkv = np.asarray(Wqkv, dtype=np.float32)
    Wout = np.asarray(Wout, dtype=np.float32)
    nc = build()
    in_maps = make_in_maps(x, Wqkv, Wout)
    res = run_bass_kernel_spmd(nc, in_maps, core_ids=list(range(8)))
    out = np.zeros((B, T, DIM), np.float32)
    for c in range(8):
        out[c // 4] += res.results[c]["y"].astype(np.float32)
    return out


# revision 89
# speedup vs baseline: 1.1814x; 1.0042x over previous
"""Trainium2 Bass kernel for block-causal (chunked) multi-head attention.

Computes, for x:[2,2048,1024], Wqkv:[3072,1024], Wout:[1024,1024]:
    qkv = x @ Wqkv.T ; per-head scaled scores; block-causal mask
    (causal OR same 64-chunk == full attention to all chunks <= own chunk);
    softmax; out = attn @ v ; y = out @ Wout.T

Sharding over 8 NeuronCores: data-parallel over batch (2) x tensor-parallel
over heads (16 heads -> 4 per core).  Each core projects q/k/v for its 4
heads, runs attention, and computes a partial output projection against its
256 columns of Wout; the host sums the 4 partials per batch element.

All SBUF operands are float16 (PE runs f16 at 1 row/cycle with no small-tile
penalty, DMA bytes halve, and the ~1e-3 quantization error is far inside the
tolerance); PSUM accumulation stays f32.

On-chip layout avoids all transposes: the host hands each core
  xT     [1024, 2048]  (x[b] transposed, f16)
  wqkT   [1024, 512]   (Wqkv rows for its 4 heads' q,k -> transposed)
  wvT    [1024, 256]   (v rows transposed)
  woutT  [256, 1024]   (Wout columns for its head-slice, transposed)
Scores are computed transposed (S^T[tk, tq]) so that the attention matmul
P^T -> (attn @ V) needs no transposes, and the softmax denominator comes
for free from a ones-column appended to V.  The block-causal mask is
realized structurally: masked-out key blocks are simply never computed, and
the diagonal blocks use rectangular sub-views (chunk granularity 64).

Engines execute their instruction streams in order, so the emission is a
software pipeline over the 4 query tiles: the TensorE stream for the
(ScalarE-paced) attention of tile t is interleaved with "filler" matmul
chains -- the output projection of tile t-1, the q projections of tile t+1,
and (deadline-scheduled) tile t's OWN k/v projections, which are only
consumed from key-block 4t onward -- keeping the PE busy through every exp
dependency stall.

Head and tail are flattened further: the first projection runs its four
chains kb-interleaved across four PSUM accumulators so the PE is never
queued behind one chain's wait for an input-DMA chunk; the softmax
normalizes of the last tiles are staggered into the attention loop (column
ranges finalize early because later diagonal blocks touch only later
columns); and during the drain the score-PSUM banks are recycled for the
output-projection pieces, whose first accumulation half runs inside the
final normalize window.
"""

import sys

if "/opt/trn_rl_repo" not in sys.path:
    sys.path.insert(0, "/opt/trn_rl_repo")

from collections import deque

import numpy as np

import concourse.bass as bass  # noqa: F401  (registers types)
import concourse.mybir as mybir
import concourse.tile as tile
from concourse import bacc
from concourse.bass_utils import run_bass_kernel_spmd

F32 = mybir.dt.float32
F16 = mybir.dt.float16
EXP = mybir.ActivationFunctionType.Exp

B = 2
T = 2048
DIM = 1024
N_HEADS = 16
HD = 64
CHUNK = 64
H_PER_CORE = 4  # 16 heads / (8 cores / 2 batches)
QT = 512  # query tile (free dim of S^T matmuls)
KB = 128  # key block (contraction block of AV matmuls)
N_QT = T // QT  # 4
N_KB = T // KB  # 16
N_DIMB = DIM // 128  # 8 contraction blocks for the projections
SCALE = 1.0 / np.sqrt(HD)

_CACHED_NC = None


def _emit(nc, tc, xT, wqkT, wvT, woT, y):
    po = tc.tile_pool  # shorthand

    with (
        po(name="persist", bufs=1) as pp,
        po(name="s_ps", bufs=2, space="PSUM") as sps,  # [128,1024] score slots
        po(name="mm_ps", bufs=2, space="PSUM") as mmps,  # [128,512] proj/y slots
        po(name="ot_ps", bufs=2, space="PSUM") as otps,  # [128,512] outT slots
        po(name="pbuf", bufs=6) as ppool,  # exp(S^T) tiles
        po(name="osbuf", bufs=2) as ospool,  # assembled normalized outT
        po(name="rbuf", bufs=2) as rpool,  # reciprocal denominators
        po(name="ybuf", bufs=6) as ypool,
    ):
        # ---- persistent SBUF tensors (kb stacked in the free dim so input
        # DMAs batch into a few large transfers) ----
        xt = [pp.tile([128, N_DIMB, QT], F16, tag=f"xt{c}", name=f"xt{c}") for c in range(N_QT)]
        wqk = pp.tile([128, N_DIMB, 512], F16, tag="wqk", name="wqk")
        wv = pp.tile([128, N_DIMB, 256], F16, tag="wv", name="wv")
        wo = pp.tile([128, 2, DIM], F16, tag="wo", name="wo")
        # q/k head-dim-major: partition block hp holds heads (2hp, 2hp+1)
        qt = [
            [pp.tile([128, QT], F16, tag=f"qt{i}_{c}", name=f"qt{i}_{c}") for c in range(N_QT)]
            for i in range(2)
        ]
        kt = [
            [pp.tile([128, QT], F16, tag=f"kt{i}_{c}", name=f"kt{i}_{c}") for c in range(N_QT)]
            for i in range(2)
        ]
        # v (token-major) + ones columns, per key block: [128, 4 heads, 2*64]
        vh = [
            pp.tile([128, H_PER_CORE, 2 * HD], F16, tag=f"vh{b}", name=f"vh{b}")
            for b in range(N_KB)
        ]

        # ---- input DMAs: tile-0 inputs arrive in fine chunks so the first
        # projection chains start early; the rest are single batched DMAs
        def src3(t, rows, cols):  # [rows*128, cols] -> [128, rows-chunks, cols]
            return t.rearrange("(k p) n -> p k n", p=128)

        chunks = [(0, 1), (1, 3), (3, 5), (5, 8)]
        for k0, k1 in chunks:  # fine pacing for wqk + xt tile 0
            nc.sync.dma_start(
                wqk[:, k0:k1, :], src3(wqkT[128 * k0 : 128 * k1, :], k1 - k0, 512)
            )
            nc.sync.dma_start(
                xt[0][:, k0:k1, :], src3(xT[128 * k0 : 128 * k1, 0:QT], k1 - k0, QT)
            )
        nc.sync.dma_start(wv[:], src3(wvT, N_DIMB, 256))
        for half in range(2):  # xt tile 1 halved: its q chains fill attend(0)
            ks = slice(4 * half, 4 * half + 4)
            nc.sync.dma_start(
                xt[1][:, ks, :],
                src3(xT[512 * half : 512 * half + 512, QT : 2 * QT], 4, QT),
            )
        for half in range(2):
            ks = slice(4 * half, 4 * half + 4)
            nc.sync.dma_start(
                xt[2][:, ks, :],
                src3(xT[512 * half : 512 * half + 512, 2 * QT : 3 * QT], 4, QT),
            )
        nc.sync.dma_start(wo[:], src3(woT, 2, DIM))
        for half in range(2):
            ks = slice(4 * half, 4 * half + 4)
            nc.sync.dma_start(
                xt[3][:, ks, :],
                src3(xT[512 * half : 512 * half + 512, 3 * QT : 4 * QT], 4, QT),
            )

        # the ones-columns of every vh tile have no input dependency:
        # emit them all at t=0 while the DVE is otherwise idle
        for tb in range(N_KB):
            nc.vector.memset(vh[tb][:, :, HD : 2 * HD], 1.0)

        def qk_chain(tt, ob):  # ob 0,1 -> q pair blocks; 2,3 -> k pair blocks
            ps = mmps.tile([128, 512], F32, tag="mm512", name=f"qk_ps{tt}_{ob}")
            for kb in range(N_DIMB):
                nc.tensor.matmul(
                    ps[:],
                    wqk[:, kb, ob * 128 : (ob + 1) * 128],
                    xt[tt][:, kb, :],
                    start=(kb == 0),
                    stop=(kb == N_DIMB - 1),
                )
            dest = (qt if ob < 2 else kt)[ob % 2][tt]
            nc.vector.tensor_copy(dest[:], ps[:])

        def v_chain(tb):
            ps = mmps.tile([128, 256], F32, tag="mm512", name=f"v_ps{tb}")
            for kb in range(N_DIMB):
                nc.tensor.matmul(
                    ps[:],
                    xt[tb // 4][:, kb, (tb % 4) * KB : (tb % 4 + 1) * KB],
                    wv[:, kb, :],
                    start=(kb == 0),
                    stop=(kb == N_DIMB - 1),
                )
            nc.vector.tensor_copy(vh[tb][:, :, 0:HD], ps[:])

        def q_pieces(tt):  # q projections: needed before attend(tt) starts
            for ob in range(2):
                yield None, (lambda ob=ob: qk_chain(tt, ob))

        def kv_pieces(tt):
            """k/v projections of tile tt, with deadlines (hp, key-block)
            inside attend(tt) itself: k for head-pair hp is first consumed by
            the S matmul of block 4*tt of that pair; v[b] by the AV matmul of
            block b of pair 0."""
            yield (0, 4 * tt), (lambda: qk_chain(tt, 2))
            for tb in range(4 * tt, 4 * tt + 4):
                # +0.5: v[tb] is consumed by the AV of block tb, which is
                # emitted after the flush point of iteration tb -- so it
                # need not precede the S pair of block tb
                yield (0, tb + 1.5), (lambda tb=tb: v_chain(tb))
            yield (1, 4 * tt), (lambda: qk_chain(tt, 3))

        def attend(tt, os_pair, fillers, deferred, reserve=0, post_hp1=None):
            nb = 4 * (tt + 1)  # allowed key blocks for this query tile
            step = 0
            done_fill = 0
            n_fill = len(fillers) - reserve

            def run_piece():
                nonlocal done_fill
                fillers.popleft()[1]()
                done_fill += 1

            def fill():
                want = min(((step + 1) * n_fill + nb) // (2 * nb), n_fill)
                while done_fill < want and fillers:
                    run_piece()

            def flush(hp, b):
                # force-run any deadline piece due at or before (hp, b)
                nonlocal done_fill
                if not any(k is not None and k <= (hp, b) for k, _ in fillers):
                    return
                rest = deque()
                while fillers:
                    k, fn = fillers.popleft()
                    if k is not None and k <= (hp, b):
                        fn()
                        done_fill += 1
                    else:
                        rest.append((k, fn))
                fillers.extend(rest)

            for hp in range(2):  # head pair (2hp, 2hp+1)
                ot = [
                    otps.tile([128, QT], F32, tag="ot", name=f"ot{tt}_{hp}_{i}")
                    for i in range(2)
                ]

                # normalize: os_pair[hp][i*64:(i+1)*64] = ot[i][0:64]/ot[i][64]
                # (denominator replicated on partitions 64-127 by the 64
                # ones-columns in vh; DVE ops partition-shift as needed)
                def norm(c0, c1, hp=hp, ot=ot):
                    for i in range(2):
                        rb = rpool.tile(
                            [64, c1 - c0],
                            F32,
                            tag=f"rb{c1 - c0}",
                            name=f"rb{tt}_{hp}_{i}_{c0}",
                        )
                        nc.vector.reciprocal(rb[:], ot[i][64:128, c0:c1])
                        nc.vector.tensor_mul(
                            os_pair[hp][i * 64 : (i + 1) * 64, c0:c1],
                            ot[i][0:64, c0:c1],
                            rb[:],
                        )

                norm_mid = lambda: norm(0, QT // 2)

                def s_mm(b):
                    """S^T for key block b, both heads, into one 2-bank tile."""
                    diag = b - 4 * tt
                    d = diag * 128 if diag >= 0 else 0
                    s = sps.tile([128, 2 * QT], F32, tag="s2", name=f"s{tt}_{hp}_{b}")
                    for i in range(2):
                        rows = slice(i * 64, i * 64 + 64)
                        nc.tensor.matmul(
                            s[:, i * QT + d : (i + 1) * QT],
                            kt[hp][b // 4][rows, (b % 4) * KB : (b % 4 + 1) * KB],
                            qt[hp][tt][rows, d:QT],
                            start=True,
                            stop=True,
                        )
                    p = ppool.tile([128, 2 * QT], F16, tag="p", name=f"p{tt}_{hp}_{b}")
                    return s, p

                flush(hp, 0)
                s_tiles = {0: s_mm(0)}
                for b in range(nb):
                    if b + 1 < nb:
                        flush(hp, b + 1)
                        s_tiles[b + 1] = s_mm(b + 1)
                    diag = b - 4 * tt
                    d = diag * 128 if diag >= 0 else 0
                    s, p = s_tiles.pop(b)
                    if diag < 0:
                        nc.scalar.activation(p[:], s[:], EXP, scale=SCALE)
                    else:
                        # one exp for both heads over cols >= d (all rows),
                        # then zero the masked corner (rows 64-127 of each
                        # head attend only cols >= d+64) AFTER the exp
                        s2 = s[:].rearrange("p (h c) -> p h c", h=2)
                        p2 = p[:].rearrange("p (h c) -> p h c", h=2)
                        nc.scalar.activation(
                            p2[:, :, d:QT], s2[:, :, d:QT], EXP, scale=SCALE
                        )
                        # keep the corner memset off the DVE stream for the
                        # last two blocks of the final pair, where it must
                        # not queue behind the staggered normalize
                        if tt >= N_QT - 2 and b >= nb - 2:
                            nc.gpsimd.memset(p2[64:128, :, d : d + 64], 0.0)
                        else:
                            nc.vector.memset(p2[64:128, :, d : d + 64], 0.0)
                    # fillers go ahead of the AV pair: the PE stream is
                    # in-order, so work emitted after a stalled AV could
                    # never overtake it during the exp's tail
                    fill()
                    flush(hp, b + 1.5)
                    for i in range(2):
                        nc.tensor.matmul(
                            ot[i][:, d:QT],
                            vh[b][:, 2 * hp + i, :],
                            p[:, i * QT + d : (i + 1) * QT],
                            start=(b == 0),
                            stop=(b == nb - 1),
                        )
                    step += 1
                    if tt >= N_QT - 2 and b == nb - 3:
                        # columns [0:256) of ot are final (the last two,
                        # diagonal, blocks only touch cols >= 256): normalize
                        # them now, overlapping the last attention steps
                        norm_mid()

                # the last normalize is staggered: cols [0:256) were
                # emitted mid-loop, cols [256:512) are deferred into the
                # drain so the first y pieces' DVE copies aren't queued
                # behind the whole chain
                if tt == N_QT - 1 and hp == 1:
                    deferred.append(lambda: norm(QT // 2, QT))
                else:
                    norm(QT // 2, QT) if tt >= N_QT - 2 else norm(0, QT)

                if tt == N_QT - 1 and hp == 1:
                    post_hp1()

            while fillers:
                run_piece()

        def y_pieces_paired(tt, os_pair, tail=False, ys2_cell=None):
            pieces = []
            # during the drain the score PSUM banks are free the moment the
            # last exp has read them: two 2-bank score tiles host four of the
            # eight pieces, so no piece's matmuls ever wait on a prior copy
            ys2 = ys2_cell[0] if tail else None
            for t4 in range(4):
                trows = slice(t4 * 128, (t4 + 1) * 128)
                ysb = ypool.tile([128, DIM], F16, tag="ysb", name=f"ysb{tt}_{t4}")
                for jb in range(2):

                    def piece(t4=t4, jb=jb, ysb=ysb, trows=trows):
                        k = t4 * 2 + jb
                        s2slot = {1: (0, 0), 2: (0, 1), 4: (1, 0), 5: (1, 1)}
                        if tail and k in s2slot:
                            # head-pair-0 half was pre-accumulated in post_hp1
                            j, h = s2slot[k]
                            yps = ys2[j][:, h * 512 : (h + 1) * 512]
                            nc.tensor.matmul(
                                yps[:],
                                os_pair[1][:, trows],
                                wo[:, 1, jb * 512 : (jb + 1) * 512],
                                start=False,
                                stop=True,
                            )
                        else:
                            yps = mmps.tile(
                                [128, 512], F32, tag="mm512", name=f"y_ps{tt}_{t4}_{jb}"
                            )
                            for db in range(2):
                                nc.tensor.matmul(
                                    yps[:],
                                    os_pair[db][:, trows],
                                    wo[:, db, jb * 512 : (jb + 1) * 512],
                                    start=(db == 0),
                                    stop=(db == 1),
                                )
                        dest = ysb[:, jb * 512 : (jb + 1) * 512]
                        rows = slice(tt * QT + t4 * 128, tt * QT + (t4 + 1) * 128)
                        if tail:
                            # drain phase: alternate the PSUM->SBUF copies
                            # between the (idle) ScalarE and the DVE so the
                            # final copy chain halves
                            if k < 4 or k == 5:
                                nc.scalar.copy(dest, yps[:])
                            else:
                                nc.vector.tensor_copy(dest, yps[:])
                        else:
                            nc.vector.tensor_copy(dest, yps[:])
                        if jb == 1:
                            nc.sync.dma_start(y[rows, :], ysb[:])

                    pieces.append((None, piece))
            return pieces

        # ---- the pipeline ----
        # Tile 0's q/k chains run kb-interleaved across four concurrent PSUM
        # accumulators (the attention pools are untouched this early), so the
        # in-order PE stream is never blocked behind one chain's wait for the
        # next input chunk.
        pre_ps = [
            mmps.tile([128, 512], F32, tag="mm512", name=f"pre_ps{j}")
            for j in range(2)
        ] + [
            otps.tile([128, QT], F32, tag="ot", name=f"pre_ps{j + 2}")
            for j in range(2)
        ]
        pre_obs = [0, 2, 1, 3]
        for kb in range(N_DIMB):
            for j, ob in enumerate(pre_obs):
                nc.tensor.matmul(
                    pre_ps[j][:],
                    wqk[:, kb, ob * 128 : (ob + 1) * 128],
                    xt[0][:, kb, :],
                    start=(kb == 0),
                    stop=(kb == N_DIMB - 1),
                )
        for j, ob in enumerate(pre_obs):
            dest = (qt if ob < 2 else kt)[ob % 2][0]
            nc.vector.tensor_copy(dest[:], pre_ps[j][:])
        prev_b = None
        for tt in range(N_QT):
            os_pair = [
                ospool.tile([128, QT], F16, tag=f"os{i}", name=f"os{i}_{tt}")
                for i in range(2)
            ]
            nxt = deque()
            if tt + 1 < N_QT:
                nxt.extend(q_pieces(tt + 1))
            if tt:
                late = deque(kv_pieces(tt))
            else:
                late = deque(
                    [((0, tb + 1.5), (lambda tb=tb: v_chain(tb)))
                     for tb in (0, 1, 2, 3)]
                )
            b = deque(prev_b) if prev_b is not None else deque()
            fillers = deque()
            while late or b or nxt:
                if late:
                    fillers.append(late.popleft())
                if b:
                    fillers.append(b.popleft())
                if late:
                    fillers.append(late.popleft())
                if nxt:
                    fillers.append(nxt.popleft())
            deferred = []
            tail = tt == N_QT - 1
            ys2_cell = [None]

            def post_hp1(os_pair=os_pair, ys2_cell=ys2_cell):
                # the score PSUM ring is done with allocations: claim both
                # tiles for the tail y pieces and pre-run their first
                # (head-pair-0) accumulation half inside the normalize
                # window, when the PE would otherwise idle
                ys2_cell[0] = [
                    sps.tile([128, 2 * QT], F32, tag="s2", name=f"ys2_{j}")
                    for j in range(2)
                ]
                s2slot = {1: (0, 0), 2: (0, 1), 4: (1, 0), 5: (1, 1)}
                for k, (j, h) in s2slot.items():
                    t4, jb = divmod(k, 2)
                    nc.tensor.matmul(
                        ys2_cell[0][j][:, h * 512 : (h + 1) * 512],
                        os_pair[0][:, t4 * 128 : (t4 + 1) * 128],
                        wo[:, 0, jb * 512 : (jb + 1) * 512],
                        start=True,
                        stop=False,
                    )


            attend(tt, os_pair, fillers, deferred, post_hp1=post_hp1 if tail else None)
            prev_b = y_pieces_paired(tt, os_pair, tail=tail, ys2_cell=ys2_cell)
        for idx, (_, piece) in enumerate(prev_b):
            if idx == 0:
                for d in deferred:
                    d()
            piece()


def build():
    global _CACHED_NC
    if _CACHED_NC is not None:
        return _CACHED_NC
    nc = bacc.Bacc(
        "TRN2", target_bir_lowering=False, debug=False, enable_asserts=False
    )
    xT = nc.dram_tensor("xT", [DIM, T], F16, kind="ExternalInput").ap()
    wqkT = nc.dram_tensor("wqkT", [DIM, 512], F16, kind="ExternalInput").ap()
    wvT = nc.dram_tensor("wvT", [DIM, 256], F16, kind="ExternalInput").ap()
    woT = nc.dram_tensor("woutT", [256, DIM], F16, kind="ExternalInput").ap()
    y = nc.dram_tensor("y", [T, DIM], F16, kind="ExternalOutput").ap()
    with tile.TileContext(nc) as tc:
        _emit(nc, tc, xT, wqkT, wvT, woT, y)
    nc.compile()
    _CACHED_NC = nc
    return nc


def make_in_maps(x, Wqkv, Wout):
    """Host-side sharding: core c = (batch c//4, head-group c%4)."""
    in_maps = []
    for c in range(8):
        b, hg = divmod(c, 4)
        hs = hg * H_PER_CORE
        r0, r1 = hs * HD, (hs + H_PER_CORE) * HD
        qrows = Wqkv[r0:r1]
        krows = Wqkv[DIM + r0 : DIM + r1]
        vrows = Wqkv[2 * DIM + r0 : 2 * DIM + r1]
        in_maps.append(
            {
                "xT": np.ascontiguousarray(x[b].T.astype(np.float16)),
                "wqkT": np.ascontiguousarray(
                    np.concatenate([qrows, krows], 0).T.astype(np.float16)
                ),
                "wvT": np.ascontiguousarray(vrows.T.astype(np.float16)),
                "woutT": np.ascontiguousarray(Wout[:, r0:r1].T.astype(np.float16)),
            }
        )
    return in_maps


def kernel(x, Wqkv, Wout):
    x = np.asarray(x, dtype=np.float32)
    Wqkv = np.asarray(Wqkv, dtype=np.float32)
    Wout = np.asarray(Wout, dtype=np.float32)
    nc = build()
    in_maps = make_in_maps(x, Wqkv, Wout)
    res = run_bass_kernel_spmd(nc, in_maps, core_ids=list(range(8)))
    out = np.zeros((B, T, DIM), np.float32)
    for c in range(8):
        out[c // 4] += res.results[c]["y"].astype(np.float32)
    return out
